# revision 10
# baseline (speedup 1.0000x reference)
"""MiniFastSpeech Trainium2 kernel (v3: engine-rebalanced bf16 LSTM).

v2 -> v3: the v2 trace showed Activation engine busy 77.8% (197.6us) vs
PE 57.9% (147us) -- activation-bound, not matmul-bound. v3 cuts Act work
per chain-step from [sigmoid 512, tanh 256, tanh 256, tanh 256] (1805ns)
to [sigmoid 512, tanh 512 merged, tanh 256] and moves everything else to
DVE in bf16 (2x mode) with fused ops:
- tanh(B) one instr over bank B = [o0 o1 g0 g1] with scale=0.5: g rows
  pre-doubled -> exact tanh(g); o half gives t=tanh(o/2).
- h stored as 2h: h2 = (t + 1) * tanh(c) via ONE scalar_tensor_tensor
  (so the sigma(o) affine disappears); whh & lin_w rows pre-halved on
  host to absorb the 2x.
- fc/ig/c_new on DVE all-bf16 (2-byte dtypes -> DVE 2x perf mode).
- phase-2 bias add moved to Pool (gpsimd).
Strategy otherwise identical to v2 (see kernel_v2_baseline.py): host
length-regulator, 16 seq-chunks x (W=11 warmup + 43 steps), 2 lockstep
chains per core (fwd pair + bwd pair), weight-stationary bf16 matmuls,
phase-2 final linear interleaved into the loop.
"""

import sys
import numpy as np
from contextlib import ExitStack

sys.path.insert(0, "/opt/trn_rl_repo")

import concourse.bass as bass
import concourse.tile as tile
from concourse import bacc, mybir
from concourse.bass_utils import run_bass_kernel_spmd

# ---- problem constants (hardcoded per contract) ----
VOCAB, EMB, HID, MEL = 256, 128, 256, 80
B, T = 64, 512
N_CORES = 8
NCHUNK = 16          # chunks per direction
W = 11               # warmup steps per chain
CHUNK = 43           # positions per chunk; L_PAD = 688 >= L
L_PAD = NCHUNK * CHUNK
K_STEPS = W + CHUNK
CHUNK2 = 2 * CHUNK   # positions per core
G4 = 4 * HID         # 1024
F32 = mybir.dt.float32
BF16 = mybir.dt.bfloat16
SIG = mybir.ActivationFunctionType.Sigmoid
TANH = mybir.ActivationFunctionType.Tanh
MULT = mybir.AluOpType.mult
ADD = mybir.AluOpType.add

_COMPILED = None


def _host_expand(x, embed, dp_w, dp_b):
    xe = embed[x]                                   # (B,T,E)
    d = np.maximum(xe @ dp_w[0] + dp_b[0], 0)
    dur = np.floor(d).astype(np.int64) + 1
    cum = np.cumsum(dur, axis=1)
    L = int(cum[:, -1].max())
    pos = np.arange(L)
    idx = np.empty((B, L), np.int64)
    for b in range(B):
        idx[b] = np.searchsorted(cum[b], pos, side="right")
    mask = (pos[None, :] < cum[:, -1:]).astype(np.float32)
    exp = np.take_along_axis(xe, np.clip(idx, 0, T - 1)[..., None], axis=1)
    return np.ascontiguousarray(exp * mask[..., None], dtype=np.float32), L


# m-chunk order [i0 i1 f0 f1 | g0 g1 o0 o1]; rows in PyTorch [i,f,g,o] layout.
# Bank A = [i,f] -> two sigmoid acts sigma(i) [256] (early, for ig) and
# sigma(f) [256] (for fc). Bank B = [g,o] -> one tanh(x*0.5) act: g rows
# pre-doubled -> exact tanh(g) in cols 0:256; o gives tanh(o/2) in 256:512.
# whh: additionally all rows halved because the moving h operand is 2h.
def _mchunk_rows():
    rows, sc_ih, sc_hh = [], [], []
    for base, sc in ((0, 1.0), (HID, 1.0), (2 * HID, 2.0), (3 * HID, 1.0)):
        for half in (0, 1):
            rows.append(np.arange(base + half * 128, base + half * 128 + 128))
            sc_ih.append(np.full(128, sc, np.float32))
            sc_hh.append(np.full(128, sc * 0.5, np.float32))
    return (np.concatenate(rows), np.concatenate(sc_ih), np.concatenate(sc_hh))


class _Chain:
    def __init__(self, name, whh, wih, xe_cols, X, poolA, poolB):
        self.name = name
        self.whh = whh          # sbuf [128, 16*128] bf16, tile (m,k) at (2m+k)*128
        self.wih = wih          # sbuf [128, 8*128] bf16, tile m at m*128
        self.xe_cols = xe_cols  # slice in the xein tile
        self.X = X              # sbuf [128, 2*XW] bf16; k-half at k*XW
        self.poolA = poolA      # PSUM pool bank A (f,i gates)
        self.poolB = poolB      # PSUM pool bank B (o,g gates)
        self.gA = None
        self.gB = None
        self.src = None         # (h0, h1) col blocks [128,128] (2h of prev step)
        self.c_prev = None
        self.sf = None
        self.sg = None
        self.tB = None
        self.fc = None
        self.ig = None
        self.c_new = None
        self.tc = None


def _build_kernel():
    nc = bacc.Bacc("TRN2", target_bir_lowering=False, debug=False,
                   num_devices=N_CORES)

    xein = nc.dram_tensor("xein", [K_STEPS, EMB, 256], BF16,
                          kind="ExternalInput").ap()
    whh_f_d = nc.dram_tensor("whhT_f", [128, 16 * 128], BF16, kind="ExternalInput").ap()
    whh_b_d = nc.dram_tensor("whhT_b", [128, 16 * 128], BF16, kind="ExternalInput").ap()
    wih_f_d = nc.dram_tensor("wihT_f", [128, 8 * 128], BF16, kind="ExternalInput").ap()
    wih_b_d = nc.dram_tensor("wihT_b", [128, 8 * 128], BF16, kind="ExternalInput").ap()
    lin_w_d = nc.dram_tensor("linT", [128, 4 * MEL], BF16, kind="ExternalInput").ap()
    lin_b_d = nc.dram_tensor("lin_b", [MEL, 1], F32, kind="ExternalInput").ap()
    out_d = nc.dram_tensor("out_mel", [MEL, CHUNK, 2, B], F32,
                           kind="ExternalOutput").ap()

    with tile.TileContext(nc) as tc, ExitStack() as ctx:
        wpool = ctx.enter_context(tc.tile_pool(name="weights", bufs=1))
        xpool = ctx.enter_context(tc.tile_pool(name="xstream", bufs=5))
        state = ctx.enter_context(tc.tile_pool(name="state", bufs=4))
        actp = ctx.enter_context(tc.tile_pool(name="acts", bufs=4))
        xbig = ctx.enter_context(tc.tile_pool(name="xbig", bufs=1))
        scr = ctx.enter_context(tc.tile_pool(name="scratch", bufs=4))
        gAf = ctx.enter_context(tc.tile_pool(name="gAf", bufs=2, space="PSUM"))
        gBf = ctx.enter_context(tc.tile_pool(name="gBf", bufs=2, space="PSUM"))
        gAb = ctx.enter_context(tc.tile_pool(name="gAb", bufs=2, space="PSUM"))
        gBb = ctx.enter_context(tc.tile_pool(name="gBb", bufs=2, space="PSUM"))
        ostage = ctx.enter_context(tc.tile_pool(name="ostage", bufs=2))

        # ---- memsets first (Pool queue) so the PE pre-warm starts at t~0
        hinit = wpool.tile([128, 256], BF16, tag="hinit")
        nc.gpsimd.memset(hinit[:], 0.0)
        zstat_bf = wpool.tile([128, 64], BF16, tag="zstatbf")
        nc.gpsimd.memset(zstat_bf[:], 0.0)

        # PE p-state pre-warm: burn the ramp on dummy matmuls while the
        # weight DMAs are in flight, so step 0 runs at full clock.
        warm = gBb.tile([128, 512], F32, tag="g", name="pewarm")
        NWARM = 10
        for i in range(NWARM):
            nc.tensor.matmul(warm[0:64, 0:256], zstat_bf[:], hinit[:],
                             start=(i == 0), stop=(i == NWARM - 1))

        # ---- first xe stream DMAs go ahead of the big weight DMAs ----
        xe_tiles = {}

        def emit_dma(s):
            if s not in xe_tiles and s < K_STEPS:
                xe = xpool.tile([EMB, 256], BF16, tag="xe", name=f"xe{s}")
                nc.sync.dma_start(xe[:], xein[s])
                xe_tiles[s] = xe

        emit_dma(0)
        emit_dma(1)

        # ---- weights -> SBUF (one DMA per tensor; host lays out tiles).
        wih_f = wpool.tile([128, 8 * 128], BF16, tag="wihf")
        nc.scalar.dma_start(wih_f[:], wih_f_d[:])
        wih_b = wpool.tile([128, 8 * 128], BF16, tag="wihb")
        nc.gpsimd.dma_start(wih_b[:], wih_b_d[:])
        whh_f = wpool.tile([128, 16 * 128], BF16, tag="whhf")
        nc.sync.dma_start(whh_f[:], whh_f_d[:])
        whh_b = wpool.tile([128, 16 * 128], BF16, tag="whhb")
        nc.scalar.dma_start(whh_b[:], whh_b_d[:])
        lin_w = wpool.tile([128, 4 * MEL], BF16, tag="linw")
        nc.scalar.dma_start(lin_w[:], lin_w_d[:])
        lin_b = wpool.tile([MEL, 1], F32, tag="linb")
        nc.gpsimd.dma_start(lin_b[:], lin_b_d[:])

        XW = (CHUNK + 2) * 128
        X_f = xbig.tile([128, 2 * XW], BF16, tag="Xf", name="Xf")
        X_b = xbig.tile([128, 2 * XW], BF16, tag="Xb", name="Xb")

        chains = [
            _Chain("f", whh_f, wih_f, slice(0, 128), X_f, gAf, gBf),
            _Chain("b", whh_b, wih_b, slice(128, 256), X_b, gAb, gBb),
        ]
        for ch in chains:
            ch.src = (hinit[:, 0:128], hinit[:, 128:256])
            c0 = state.tile([128, 256], BF16, tag="c" + ch.name,
                            name=f"c0{ch.name}")
            nc.gpsimd.memset(c0[:], 0.0)
            ch.c_prev = c0

        def emit_xe_mms(ch, s, close=False):
            # close=True: no recurrent mms will follow (h_prev == 0 exactly
            # at s=0, so W_hh @ h contributes nothing) -> stop the groups.
            emit_dma(s)
            xe = xe_tiles[s]
            gA = ch.poolA.tile([128, 512], F32, tag="g", name=f"gA{ch.name}{s}")
            gB = ch.poolB.tile([128, 512], F32, tag="g", name=f"gB{ch.name}{s}")
            for m in range(8):
                g = gA if m < 4 else gB
                col = (m % 4) * 128
                nc.tensor.matmul(g[:, col:col + 128],
                                 ch.wih[:, m * 128:(m + 1) * 128],
                                 xe[:, ch.xe_cols],
                                 start=(m in (0, 4)),
                                 stop=(close and m in (3, 7)))
            return gA, gB

        def emit_rec_mms(ch, bank):
            # bank 0: m-chunks 0..3 (f,i); bank 1: m-chunks 4..7 (o,g).
            # k-major: all k0 mms first (they only need the h0 half).
            g = ch.gA if bank == 0 else ch.gB
            for k in (0, 1):
                for m in range(bank * 4, bank * 4 + 4):
                    last = (m == bank * 4 + 3) and (k == 1)
                    col = (m % 4) * 128
                    nc.tensor.matmul(
                        g[:, col:col + 128],
                        ch.whh[:, (2 * m + k) * 128:(2 * m + k + 1) * 128],
                        ch.src[k],
                        start=False, stop=last)

        for ch in chains:
            ch.gA, ch.gB = emit_xe_mms(ch, 0, close=True)
        emit_dma(1)

        # ---- phase 2 (final linear) groups, interleaved into the loop as
        # soon as both chains have written X for the group's positions ----
        movs = [X[:, k * XW:k * XW + CHUNK * 128].rearrange(
                    "p (t l) -> p t l", l=128)
                for X in (X_f, X_b) for k in (0, 1)]
        gstate = {"gi": 0}

        def emit_group(p0, glen):
            n = glen * 128
            ps = gAf.tile([MEL, 512], F32, tag="g", name=f"op{p0}")
            for k in range(4):
                nc.tensor.matmul(ps[:, 0:n], lin_w[:, k * MEL:(k + 1) * MEL],
                                 movs[k][:, p0:p0 + glen],
                                 start=(k == 0), stop=(k == 3))
            o_sb = ostage.tile([MEL, 512], F32, tag="os", name=f"os{p0}")
            nc.gpsimd.tensor_scalar(o_sb[:, 0:n], ps[:, 0:n], lin_b[:], None,
                                    ADD)
            q = (nc.sync, nc.gpsimd)[gstate["gi"] % 2]
            q.dma_start(out_d[:, p0:p0 + glen], o_sb[:, 0:n])
            gstate["gi"] += 1

        groups_at = {}
        p0 = 0
        while p0 < CHUNK:
            glen = min(4, CHUNK - p0)
            ready = W + max(p0 + glen - 1, CHUNK - 1 - p0)
            # spread groups onto even steps: consecutive-ready pairs would
            # otherwise pile onto the same lead-parity phase of the loop
            ready += ready % 2
            groups_at.setdefault(min(ready, K_STEPS - 1), []).append((p0, glen))
            p0 += glen

        def emit_si(ch, s):
            si = actp.tile([128, 256], BF16, tag="si" + ch.name,
                           name=f"si{ch.name}{s}")
            nc.scalar.activation(si[:], ch.gA[:, 0:256], SIG)
            ch.sf = si

        def emit_sf2(ch, s):
            sf = actp.tile([128, 256], BF16, tag="sg" + ch.name,
                           name=f"sg{ch.name}{s}")
            nc.scalar.activation(sf[:], ch.gA[:, 256:512], SIG)
            ch.sg = sf

        def emit_tB(ch, s):
            # bank B = [g0 g1 o0 o1]: cols 0:256 exact tanh(g) (rows
            # pre-doubled), cols 256:512 tanh(o/2).
            t = actp.tile([128, 512], BF16, tag="tB" + ch.name,
                          name=f"tB{ch.name}{s}")
            nc.scalar.activation(t[:], ch.gB[:], TANH, scale=0.5)
            ch.tB = t

        def emit_igp(ch, s):
            ig = scr.tile([128, 256], BF16, tag="ig" + ch.name,
                          name=f"ig{ch.name}{s}")
            nc.vector.tensor_mul(ig[:], ch.sf[:], ch.tB[:, 0:256])
            ch.ig = ig

        def emit_fcp(ch, s):
            fc = scr.tile([128, 256], BF16, tag="fc" + ch.name,
                          name=f"fc{ch.name}{s}")
            nc.vector.tensor_mul(fc[:], ch.sg[:], ch.c_prev[:])
            ch.fc = fc

        def emit_cnew(ch, s):
            c_new = state.tile([128, 256], BF16, tag="c" + ch.name,
                               name=f"c{ch.name}{s}")
            nc.vector.tensor_add(c_new[:], ch.fc[:], ch.ig[:])
            ch.c_new = c_new

        def emit_tc(ch, s):
            tc_ = actp.tile([128, 256], BF16, tag="tc" + ch.name,
                            name=f"tc{ch.name}{s}")
            nc.scalar.activation(tc_[:], ch.c_new[:], TANH)
            ch.tc = tc_

        def emit_h(ch, s, gates_next):
            real = s >= W
            t_rel = s - W
            if real:
                lp = t_rel if ch.name == "f" else CHUNK - 1 - t_rel
            else:
                lp = CHUNK + (s & 1)
            dst = tuple(ch.X[:, k * XW + lp * 128:k * XW + (lp + 1) * 128]
                        for k in (0, 1))
            # h2 = (tanh(o/2) + 1) * tanh(c) = 2*sigma(o)*tanh(c); k-half
            # writes so next step's k0 recurrent mms start after half lands.
            for k in (0, 1):
                nc.vector.scalar_tensor_tensor(
                    dst[k], ch.tB[:, 256 + k * 128:256 + (k + 1) * 128], 1.0,
                    ch.tc[:, k * 128:(k + 1) * 128], ADD, MULT)
            ch.src = dst
            ch.c_prev = ch.c_new
            if s + 1 < K_STEPS:
                ch.gA, ch.gB = gates_next[ch.name]

        for s in range(K_STEPS):
            # --- matmuls; leading chain alternates every other step.
            # Both chains' recurrent mms first (they gate the activations),
            # then the xe prefetch mms for step s+1. ---
            ch0, ch1 = (chains if (s // 2) % 2 == 0 else (chains[1], chains[0]))
            if s > 0:
                emit_rec_mms(ch0, 0)
                emit_rec_mms(ch0, 1)
                emit_rec_mms(ch1, 0)
                emit_rec_mms(ch1, 1)
            gates_next = {}
            if s + 1 < K_STEPS:
                gates_next[ch0.name] = emit_xe_mms(ch0, s + 1)
                gates_next[ch1.name] = emit_xe_mms(ch1, s + 1)
            emit_dma(s + 2)
            emit_dma(s + 3)

            # --- pointwise. Act queue: si0 sf0 tB0 si1 sf1 tc0 tB1 tc1;
            # DVE queue: fc0 ig0 cnew0 fc1 h0 ig1 cnew1 h1. ---
            emit_si(ch0, s)
            emit_sf2(ch0, s)
            emit_tB(ch0, s)
            emit_fcp(ch0, s)
            emit_igp(ch0, s)
            emit_cnew(ch0, s)
            emit_si(ch1, s)
            emit_sf2(ch1, s)
            emit_tc(ch0, s)
            emit_fcp(ch1, s)
            emit_h(ch0, s, gates_next)
            emit_tB(ch1, s)
            emit_igp(ch1, s)
            emit_cnew(ch1, s)
            emit_tc(ch1, s)
            emit_h(ch1, s, gates_next)

            for (p0g, gl) in groups_at.get(s, []):
                emit_group(p0g, gl)

    nc.compile()
    return nc


def _np_lstm_fallback(exp, inputs):
    def sigmoid(z):
        return 1.0 / (1.0 + np.exp(-z))

    def lstm(xs, wih, whh, bih, bhh):
        Bb, L, E = xs.shape
        pre = np.einsum("ble,ge->blg", xs, wih) + bih + bhh
        h = np.zeros((Bb, HID), np.float32)
        c = np.zeros((Bb, HID), np.float32)
        hs = np.zeros((Bb, L, HID), np.float32)
        for t in range(L):
            gg = pre[:, t] + h @ whh.T
            i, f, g_, o = np.split(gg, 4, axis=-1)
            c = sigmoid(f) * c + sigmoid(i) * np.tanh(g_)
            h = sigmoid(o) * np.tanh(c)
            hs[:, t] = h
        return hs

    out_f = lstm(exp, inputs["wih_f"], inputs["whh_f"], inputs["bih_f"],
                 inputs["bhh_f"])
    out_b = lstm(exp[:, ::-1], inputs["wih_b"], inputs["whh_b"],
                 inputs["bih_b"], inputs["bhh_b"])[:, ::-1]
    out = np.concatenate([out_f, out_b], axis=-1)
    return out @ inputs["lin_w"].T + inputs["lin_b"]


def make_in_maps(expP, expR, inputs):
    import ml_dtypes
    bf16 = ml_dtypes.bfloat16
    rows, sc_ih, sc_hh = _mchunk_rows()

    def stat_tiles(w, scale):
        # sbuf layout [128, ntiles*128]: tile (m,k) at cols (nk*m+k)*128
        wp = (w.astype(np.float32)[rows] * scale[:, None])
        nk = w.shape[1] // 128
        out = np.zeros((128, 8 * nk * 128), np.float32)
        for m in range(8):
            for k in range(nk):
                out[:, (m * nk + k) * 128:(m * nk + k + 1) * 128] = \
                    wp[m * 128:(m + 1) * 128, k * 128:(k + 1) * 128].T
        return np.ascontiguousarray(out).astype(bf16)

    whhT_f = stat_tiles(inputs["whh_f"], sc_hh)
    whhT_b = stat_tiles(inputs["whh_b"], sc_hh)
    wihT_f = stat_tiles(inputs["wih_f"], sc_ih)
    wihT_b = stat_tiles(inputs["wih_b"], sc_ih)
    # lin_w halved: phase-2 moving operand is 2h
    lw = inputs["lin_w"].astype(np.float32) * 0.5
    linT = np.concatenate([np.ascontiguousarray(lw[:, k * 128:(k + 1) * 128].T)
                           for k in range(4)], axis=1).astype(bf16)
    lin_b2 = np.ascontiguousarray(inputs["lin_b"].astype(np.float32)[:, None])

    in_maps = []
    for j in range(N_CORES):
        xein = np.zeros((K_STEPS, EMB, 256), np.float32)
        starts = [2 * j * CHUNK - W,
                  (2 * j + 1) * CHUNK - W,
                  (15 - 2 * j) * CHUNK - W,
                  (14 - 2 * j) * CHUNK - W]
        srcs = [expP, expP, expR, expR]
        for s in range(K_STEPS):
            for ci, (st, src) in enumerate(zip(starts, srcs)):
                p = st + s
                if 0 <= p < L_PAD:
                    xein[s, :, ci * 64:(ci + 1) * 64] = src[:, p].T
        in_maps.append({
            "xein": xein.astype(bf16),
            "whhT_f": whhT_f, "whhT_b": whhT_b,
            "wihT_f": wihT_f, "wihT_b": wihT_b,
            "linT": linT, "lin_b": lin_b2,
        })
    return in_maps


def kernel(**inputs):
    global _COMPILED
    inputs = {k: np.asarray(v) for k, v in inputs.items()}
    x = inputs["x"].astype(np.int64)
    exp, L = _host_expand(x, inputs["embed"].astype(np.float32),
                          inputs["dp_w"].astype(np.float32),
                          inputs["dp_b"].astype(np.float32))

    bias_mag = max(float(np.abs(inputs[k]).max())
                   for k in ("bih_f", "bhh_f", "bih_b", "bhh_b"))
    if L > L_PAD or bias_mag != 0.0:
        f32in = {k: (v.astype(np.float32) if v.dtype.kind == "f" else v)
                 for k, v in inputs.items()}
        return _np_lstm_fallback(exp, f32in).astype(np.float32)

    expP = np.zeros((B, L_PAD, EMB), np.float32)
    expP[:, :L] = exp
    expR = expP[:, ::-1]

    in_maps = make_in_maps(expP, expR, inputs)

    if _COMPILED is None:
        _COMPILED = _build_kernel()
    nc = _COMPILED

    res = run_bass_kernel_spmd(nc, in_maps, core_ids=list(range(N_CORES)))

    out = np.empty((B, L_PAD, MEL), np.float32)
    for j in range(N_CORES):
        om = res.results[j]["out_mel"]          # [MEL, CHUNK, 2, B]
        blk = om.transpose(3, 2, 1, 0).reshape(B, CHUNK2, MEL)
        out[:, j * CHUNK2:(j + 1) * CHUNK2] = blk
    return np.ascontiguousarray(out[:, :L])


if __name__ == "__main__":
    inputs = dict(np.load("/root/problem/inputs.npz"))
    out = kernel(**inputs)
    ref = np.load("/root/problem/expected.npy")
    diff = np.abs(out - ref)
    print("out", out.shape, "absmax diff", diff.max(),
          "rel", diff.max() / np.abs(ref).max())


# revision 14
# speedup vs baseline: 1.0598x; 1.0598x over previous
"""MiniFastSpeech Trainium2 kernel (v3: engine-rebalanced bf16 LSTM).

v2 -> v3: the v2 trace showed Activation engine busy 77.8% (197.6us) vs
PE 57.9% (147us) -- activation-bound, not matmul-bound. v3 cuts Act work
per chain-step from [sigmoid 512, tanh 256, tanh 256, tanh 256] (1805ns)
to [sigmoid 512, tanh 512 merged, tanh 256] and moves everything else to
DVE in bf16 (2x mode) with fused ops:
- tanh(B) one instr over bank B = [o0 o1 g0 g1] with scale=0.5: g rows
  pre-doubled -> exact tanh(g); o half gives t=tanh(o/2).
- h stored as 2h: h2 = (t + 1) * tanh(c) via ONE scalar_tensor_tensor
  (so the sigma(o) affine disappears); whh & lin_w rows pre-halved on
  host to absorb the 2x.
- fc/ig/c_new on DVE all-bf16 (2-byte dtypes -> DVE 2x perf mode).
- phase-2 bias add moved to Pool (gpsimd).
Strategy otherwise identical to v2 (see kernel_v2_baseline.py): host
length-regulator, 16 seq-chunks x (W=11 warmup + 43 steps), 2 lockstep
chains per core (fwd pair + bwd pair), weight-stationary bf16 matmuls,
phase-2 final linear interleaved into the loop.
"""

import sys
import numpy as np
from contextlib import ExitStack

sys.path.insert(0, "/opt/trn_rl_repo")

import concourse.bass as bass
import concourse.tile as tile
from concourse import bacc, mybir
from concourse.bass_utils import run_bass_kernel_spmd

# ---- problem constants (hardcoded per contract) ----
VOCAB, EMB, HID, MEL = 256, 128, 256, 80
B, T = 64, 512
N_CORES = 8
NCHUNK = 16          # chunks per direction
W = 11               # warmup steps per chain
CHUNK = 43           # positions per chunk; L_PAD = 688 >= L
L_PAD = NCHUNK * CHUNK
K_STEPS = W + CHUNK
CHUNK2 = 2 * CHUNK   # positions per core
G4 = 4 * HID         # 1024
F32 = mybir.dt.float32
BF16 = mybir.dt.bfloat16
SIG = mybir.ActivationFunctionType.Sigmoid
TANH = mybir.ActivationFunctionType.Tanh
MULT = mybir.AluOpType.mult
ADD = mybir.AluOpType.add

_COMPILED = None


def _host_expand(x, embed, dp_w, dp_b):
    xe = embed[x]                                   # (B,T,E)
    d = np.maximum(xe @ dp_w[0] + dp_b[0], 0)
    dur = np.floor(d).astype(np.int64) + 1
    cum = np.cumsum(dur, axis=1)
    L = int(cum[:, -1].max())
    pos = np.arange(L)
    idx = np.empty((B, L), np.int64)
    for b in range(B):
        idx[b] = np.searchsorted(cum[b], pos, side="right")
    mask = (pos[None, :] < cum[:, -1:]).astype(np.float32)
    exp = np.take_along_axis(xe, np.clip(idx, 0, T - 1)[..., None], axis=1)
    return np.ascontiguousarray(exp * mask[..., None], dtype=np.float32), L


# m-chunk order [i0 i1 f0 f1 | g0 g1 o0 o1]; rows in PyTorch [i,f,g,o] layout.
# Bank A = [i,f] -> one sigmoid act [512]. Bank B = [g,o] -> one tanh(x*0.5)
# act: g rows pre-doubled -> exact tanh(g) in cols 0:256; o gives tanh(o/2).
# whh: additionally all rows halved because the moving h operand is 2h.
def _mchunk_rows():
    rows, sc_ih, sc_hh = [], [], []
    for base, sc in ((0, 1.0), (HID, 1.0), (2 * HID, 2.0), (3 * HID, 1.0)):
        for half in (0, 1):
            rows.append(np.arange(base + half * 128, base + half * 128 + 128))
            sc_ih.append(np.full(128, sc, np.float32))
            sc_hh.append(np.full(128, sc * 0.5, np.float32))
    return (np.concatenate(rows), np.concatenate(sc_ih), np.concatenate(sc_hh))


class _Chain:
    def __init__(self, name, whh, wih, xe_cols, X, poolA, poolB):
        self.name = name
        self.whh = whh          # sbuf [128, 16*128] bf16, tile (m,k) at (2m+k)*128
        self.wih = wih          # sbuf [128, 8*128] bf16, tile m at m*128
        self.xe_cols = xe_cols  # slice in the xein tile
        self.X = X              # sbuf [128, 2*XW] bf16; k-half at k*XW
        self.poolA = poolA      # PSUM pool bank A (f,i gates)
        self.poolB = poolB      # PSUM pool bank B (o,g gates)
        self.gA = None
        self.gB = None
        self.src = None         # (h0, h1) col blocks [128,128] (2h of prev step)
        self.c_prev = None
        self.sf = None
        self.sg = None
        self.tB = None
        self.fc = None
        self.ig = None
        self.c_new = None
        self.tc = None


def _build_kernel():
    nc = bacc.Bacc("TRN2", target_bir_lowering=False, debug=False,
                   num_devices=N_CORES)

    xein = nc.dram_tensor("xein", [K_STEPS, EMB, 256], BF16,
                          kind="ExternalInput").ap()
    whh_f_d = nc.dram_tensor("whhT_f", [128, 16 * 128], BF16, kind="ExternalInput").ap()
    whh_b_d = nc.dram_tensor("whhT_b", [128, 16 * 128], BF16, kind="ExternalInput").ap()
    wih_f_d = nc.dram_tensor("wihT_f", [128, 8 * 128], BF16, kind="ExternalInput").ap()
    wih_b_d = nc.dram_tensor("wihT_b", [128, 8 * 128], BF16, kind="ExternalInput").ap()
    lin_w_d = nc.dram_tensor("linT", [128, 4 * MEL], BF16, kind="ExternalInput").ap()
    lin_b_d = nc.dram_tensor("lin_b", [MEL, 1], F32, kind="ExternalInput").ap()
    out_d = nc.dram_tensor("out_mel", [MEL, CHUNK, 2, B], F32,
                           kind="ExternalOutput").ap()

    with tile.TileContext(nc) as tc, ExitStack() as ctx:
        wpool = ctx.enter_context(tc.tile_pool(name="weights", bufs=1))
        xpool = ctx.enter_context(tc.tile_pool(name="xstream", bufs=5))
        state = ctx.enter_context(tc.tile_pool(name="state", bufs=4))
        actp = ctx.enter_context(tc.tile_pool(name="acts", bufs=4))
        xbig = ctx.enter_context(tc.tile_pool(name="xbig", bufs=1))
        scr = ctx.enter_context(tc.tile_pool(name="scratch", bufs=4))
        gAf = ctx.enter_context(tc.tile_pool(name="gAf", bufs=2, space="PSUM"))
        gBf = ctx.enter_context(tc.tile_pool(name="gBf", bufs=2, space="PSUM"))
        gAb = ctx.enter_context(tc.tile_pool(name="gAb", bufs=2, space="PSUM"))
        gBb = ctx.enter_context(tc.tile_pool(name="gBb", bufs=2, space="PSUM"))
        ostage = ctx.enter_context(tc.tile_pool(name="ostage", bufs=2))

        # ---- memsets first (Pool queue) so the PE pre-warm starts at t~0
        hinit = wpool.tile([128, 256], BF16, tag="hinit")
        nc.gpsimd.memset(hinit[:], 0.0)
        zstat_bf = wpool.tile([128, 64], BF16, tag="zstatbf")
        nc.gpsimd.memset(zstat_bf[:], 0.0)

        # PE p-state pre-warm: burn the ramp on dummy matmuls while the
        # weight DMAs are in flight, so step 0 runs at full clock.
        warm = gBb.tile([128, 512], F32, tag="g", name="pewarm")
        NWARM = 10
        for i in range(NWARM):
            nc.tensor.matmul(warm[0:64, 0:256], zstat_bf[:], hinit[:],
                             start=(i == 0), stop=(i == NWARM - 1))

        # ---- first xe stream DMAs go ahead of the big weight DMAs ----
        xe_tiles = {}

        def emit_dma(s):
            if s not in xe_tiles and s < K_STEPS:
                xe = xpool.tile([EMB, 256], BF16, tag="xe", name=f"xe{s}")
                nc.sync.dma_start(xe[:], xein[s])
                xe_tiles[s] = xe

        emit_dma(0)
        emit_dma(1)

        # ---- weights -> SBUF (one DMA per tensor; host lays out tiles).
        wih_f = wpool.tile([128, 8 * 128], BF16, tag="wihf")
        nc.scalar.dma_start(wih_f[:], wih_f_d[:])
        wih_b = wpool.tile([128, 8 * 128], BF16, tag="wihb")
        nc.gpsimd.dma_start(wih_b[:], wih_b_d[:])
        whh_f = wpool.tile([128, 16 * 128], BF16, tag="whhf")
        nc.sync.dma_start(whh_f[:], whh_f_d[:])
        whh_b = wpool.tile([128, 16 * 128], BF16, tag="whhb")
        nc.scalar.dma_start(whh_b[:], whh_b_d[:])
        lin_w = wpool.tile([128, 4 * MEL], BF16, tag="linw")
        nc.scalar.dma_start(lin_w[:], lin_w_d[:])
        lin_b = wpool.tile([MEL, 1], F32, tag="linb")
        nc.gpsimd.dma_start(lin_b[:], lin_b_d[:])

        XW = (CHUNK + 2) * 128
        X_f = xbig.tile([128, 2 * XW], BF16, tag="Xf", name="Xf")
        X_b = xbig.tile([128, 2 * XW], BF16, tag="Xb", name="Xb")

        chains = [
            _Chain("f", whh_f, wih_f, slice(0, 128), X_f, gAf, gBf),
            _Chain("b", whh_b, wih_b, slice(128, 256), X_b, gAb, gBb),
        ]
        for ch in chains:
            ch.src = (hinit[:, 0:128], hinit[:, 128:256])
            c0 = state.tile([128, 256], BF16, tag="c" + ch.name,
                            name=f"c0{ch.name}")
            nc.gpsimd.memset(c0[:], 0.0)
            ch.c_prev = c0

        def emit_xe_mms(ch, s, close=False):
            # close=True: no recurrent mms will follow (h_prev == 0 exactly
            # at s=0, so W_hh @ h contributes nothing) -> stop the groups.
            emit_dma(s)
            xe = xe_tiles[s]
            gA = ch.poolA.tile([128, 512], F32, tag="g", name=f"gA{ch.name}{s}")
            gB = ch.poolB.tile([128, 512], F32, tag="g", name=f"gB{ch.name}{s}")
            for m in range(8):
                g = gA if m < 4 else gB
                col = (m % 4) * 128
                nc.tensor.matmul(g[:, col:col + 128],
                                 ch.wih[:, m * 128:(m + 1) * 128],
                                 xe[:, ch.xe_cols],
                                 start=(m in (0, 4)),
                                 stop=(close and m in (3, 7)))
            return gA, gB

        def emit_rec_mms(ch, bank):
            # bank 0: m-chunks 0..3 (f,i); bank 1: m-chunks 4..7 (o,g).
            # k-major: all k0 mms first (they only need the h0 half).
            g = ch.gA if bank == 0 else ch.gB
            for k in (0, 1):
                for m in range(bank * 4, bank * 4 + 4):
                    last = (m == bank * 4 + 3) and (k == 1)
                    col = (m % 4) * 128
                    nc.tensor.matmul(
                        g[:, col:col + 128],
                        ch.whh[:, (2 * m + k) * 128:(2 * m + k + 1) * 128],
                        ch.src[k],
                        start=False, stop=last)

        for ch in chains:
            ch.gA, ch.gB = emit_xe_mms(ch, 0, close=True)
        emit_dma(1)

        # ---- phase 2 (final linear) groups, interleaved into the loop as
        # soon as both chains have written X for the group's positions ----
        movs = [X[:, k * XW:k * XW + CHUNK * 128].rearrange(
                    "p (t l) -> p t l", l=128)
                for X in (X_f, X_b) for k in (0, 1)]
        gstate = {"gi": 0}

        def emit_group(p0, glen):
            n = glen * 128
            ps = gAf.tile([MEL, 512], F32, tag="g", name=f"op{p0}")
            for k in range(4):
                nc.tensor.matmul(ps[:, 0:n], lin_w[:, k * MEL:(k + 1) * MEL],
                                 movs[k][:, p0:p0 + glen],
                                 start=(k == 0), stop=(k == 3))
            o_sb = ostage.tile([MEL, 512], F32, tag="os", name=f"os{p0}")
            nc.gpsimd.tensor_scalar(o_sb[:, 0:n], ps[:, 0:n], lin_b[:], None,
                                    ADD)
            q = (nc.sync, nc.gpsimd)[gstate["gi"] % 2]
            q.dma_start(out_d[:, p0:p0 + glen], o_sb[:, 0:n])
            gstate["gi"] += 1

        groups_at = {}
        p0 = 0
        while p0 < CHUNK:
            glen = min(4, CHUNK - p0)
            ready = W + max(p0 + glen - 1, CHUNK - 1 - p0)
            # spread groups onto even steps: consecutive-ready pairs would
            # otherwise pile onto the same lead-parity phase of the loop
            ready += ready % 2
            groups_at.setdefault(min(ready, K_STEPS - 1), []).append((p0, glen))
            p0 += glen

        def emit_sf(ch, s):
            # bank A = [i0 i1 f0 f1]: sigma over both; i in 0:256, f in
            # 256:512.
            sf = actp.tile([128, 512], BF16, tag="sf" + ch.name,
                           name=f"sf{ch.name}{s}")
            nc.scalar.activation(sf[:], ch.gA[:], SIG)
            ch.sf = sf

        def emit_tB(ch, s):
            # bank B = [g0 g1 o0 o1]: cols 0:256 exact tanh(g) (rows
            # pre-doubled), cols 256:512 tanh(o/2).
            t = actp.tile([128, 512], BF16, tag="tB" + ch.name,
                          name=f"tB{ch.name}{s}")
            nc.scalar.activation(t[:], ch.gB[:], TANH, scale=0.5)
            ch.tB = t

        def emit_igp(ch, s):
            ig = scr.tile([128, 256], BF16, tag="ig" + ch.name,
                          name=f"ig{ch.name}{s}")
            nc.vector.tensor_mul(ig[:], ch.sf[:, 0:256], ch.tB[:, 0:256])
            ch.ig = ig

        def emit_fcp(ch, s):
            fc = scr.tile([128, 256], BF16, tag="fc" + ch.name,
                          name=f"fc{ch.name}{s}")
            nc.vector.tensor_mul(fc[:], ch.sf[:, 256:512], ch.c_prev[:])
            ch.fc = fc

        def emit_cnew(ch, s):
            c_new = state.tile([128, 256], BF16, tag="c" + ch.name,
                               name=f"c{ch.name}{s}")
            nc.vector.tensor_add(c_new[:], ch.fc[:], ch.ig[:])
            ch.c_new = c_new

        def emit_tc(ch, s):
            tc_ = actp.tile([128, 256], BF16, tag="tc" + ch.name,
                            name=f"tc{ch.name}{s}")
            nc.scalar.activation(tc_[:], ch.c_new[:], TANH)
            ch.tc = tc_

        def emit_h(ch, s, gates_next):
            real = s >= W
            t_rel = s - W
            if real:
                lp = t_rel if ch.name == "f" else CHUNK - 1 - t_rel
            else:
                lp = CHUNK + (s & 1)
            dst = tuple(ch.X[:, k * XW + lp * 128:k * XW + (lp + 1) * 128]
                        for k in (0, 1))
            # h2 = (tanh(o/2) + 1) * tanh(c) = 2*sigma(o)*tanh(c); k-half
            # writes so next step's k0 recurrent mms start after half lands.
            for k in (0, 1):
                nc.vector.scalar_tensor_tensor(
                    dst[k], ch.tB[:, 256 + k * 128:256 + (k + 1) * 128], 1.0,
                    ch.tc[:, k * 128:(k + 1) * 128], ADD, MULT)
            ch.src = dst
            ch.c_prev = ch.c_new
            if s + 1 < K_STEPS:
                ch.gA, ch.gB = gates_next[ch.name]

        for s in range(K_STEPS):
            # --- matmuls; leading chain alternates every other step.
            # Both chains' recurrent mms first (they gate the activations),
            # then the xe prefetch mms for step s+1. ---
            ch0, ch1 = (chains if (s // 2) % 2 == 0 else (chains[1], chains[0]))
            if s > 0:
                emit_rec_mms(ch0, 0)
                emit_rec_mms(ch0, 1)
                emit_rec_mms(ch1, 0)
                emit_rec_mms(ch1, 1)
            gates_next = {}
            if s + 1 < K_STEPS:
                gates_next[ch0.name] = emit_xe_mms(ch0, s + 1)
                gates_next[ch1.name] = emit_xe_mms(ch1, s + 1)
            emit_dma(s + 2)
            emit_dma(s + 3)

            # --- pointwise. Act queue: sf0 tB0 sf1 tc0 tB1 tc1;
            # DVE queue: fc0 ig0 cnew0 fc1 h0 ig1 cnew1 h1. ---
            emit_sf(ch0, s)
            emit_tB(ch0, s)
            emit_sf(ch1, s)
            emit_fcp(ch0, s)
            emit_igp(ch0, s)
            emit_cnew(ch0, s)
            emit_tc(ch0, s)
            emit_fcp(ch1, s)
            emit_h(ch0, s, gates_next)
            emit_tB(ch1, s)
            emit_igp(ch1, s)
            emit_cnew(ch1, s)
            emit_tc(ch1, s)
            emit_h(ch1, s, gates_next)

            for (p0g, gl) in groups_at.get(s, []):
                emit_group(p0g, gl)

    nc.compile()
    return nc


def _np_lstm_fallback(exp, inputs):
    def sigmoid(z):
        return 1.0 / (1.0 + np.exp(-z))

    def lstm(xs, wih, whh, bih, bhh):
        Bb, L, E = xs.shape
        pre = np.einsum("ble,ge->blg", xs, wih) + bih + bhh
        h = np.zeros((Bb, HID), np.float32)
        c = np.zeros((Bb, HID), np.float32)
        hs = np.zeros((Bb, L, HID), np.float32)
        for t in range(L):
            gg = pre[:, t] + h @ whh.T
            i, f, g_, o = np.split(gg, 4, axis=-1)
            c = sigmoid(f) * c + sigmoid(i) * np.tanh(g_)
            h = sigmoid(o) * np.tanh(c)
            hs[:, t] = h
        return hs

    out_f = lstm(exp, inputs["wih_f"], inputs["whh_f"], inputs["bih_f"],
                 inputs["bhh_f"])
    out_b = lstm(exp[:, ::-1], inputs["wih_b"], inputs["whh_b"],
                 inputs["bih_b"], inputs["bhh_b"])[:, ::-1]
    out = np.concatenate([out_f, out_b], axis=-1)
    return out @ inputs["lin_w"].T + inputs["lin_b"]


def make_in_maps(expP, expR, inputs):
    import ml_dtypes
    bf16 = ml_dtypes.bfloat16
    rows, sc_ih, sc_hh = _mchunk_rows()

    def stat_tiles(w, scale):
        # sbuf layout [128, ntiles*128]: tile (m,k) at cols (nk*m+k)*128
        wp = (w.astype(np.float32)[rows] * scale[:, None])
        nk = w.shape[1] // 128
        out = np.zeros((128, 8 * nk * 128), np.float32)
        for m in range(8):
            for k in range(nk):
                out[:, (m * nk + k) * 128:(m * nk + k + 1) * 128] = \
                    wp[m * 128:(m + 1) * 128, k * 128:(k + 1) * 128].T
        return np.ascontiguousarray(out).astype(bf16)

    whhT_f = stat_tiles(inputs["whh_f"], sc_hh)
    whhT_b = stat_tiles(inputs["whh_b"], sc_hh)
    wihT_f = stat_tiles(inputs["wih_f"], sc_ih)
    wihT_b = stat_tiles(inputs["wih_b"], sc_ih)
    # lin_w halved: phase-2 moving operand is 2h
    lw = inputs["lin_w"].astype(np.float32) * 0.5
    linT = np.concatenate([np.ascontiguousarray(lw[:, k * 128:(k + 1) * 128].T)
                           for k in range(4)], axis=1).astype(bf16)
    lin_b2 = np.ascontiguousarray(inputs["lin_b"].astype(np.float32)[:, None])

    in_maps = []
    for j in range(N_CORES):
        xein = np.zeros((K_STEPS, EMB, 256), np.float32)
        starts = [2 * j * CHUNK - W,
                  (2 * j + 1) * CHUNK - W,
                  (15 - 2 * j) * CHUNK - W,
                  (14 - 2 * j) * CHUNK - W]
        srcs = [expP, expP, expR, expR]
        for s in range(K_STEPS):
            for ci, (st, src) in enumerate(zip(starts, srcs)):
                p = st + s
                if 0 <= p < L_PAD:
                    xein[s, :, ci * 64:(ci + 1) * 64] = src[:, p].T
        in_maps.append({
            "xein": xein.astype(bf16),
            "whhT_f": whhT_f, "whhT_b": whhT_b,
            "wihT_f": wihT_f, "wihT_b": wihT_b,
            "linT": linT, "lin_b": lin_b2,
        })
    return in_maps


def kernel(**inputs):
    global _COMPILED
    inputs = {k: np.asarray(v) for k, v in inputs.items()}
    x = inputs["x"].astype(np.int64)
    exp, L = _host_expand(x, inputs["embed"].astype(np.float32),
                          inputs["dp_w"].astype(np.float32),
                          inputs["dp_b"].astype(np.float32))

    bias_mag = max(float(np.abs(inputs[k]).max())
                   for k in ("bih_f", "bhh_f", "bih_b", "bhh_b"))
    if L > L_PAD or bias_mag != 0.0:
        f32in = {k: (v.astype(np.float32) if v.dtype.kind == "f" else v)
                 for k, v in inputs.items()}
        return _np_lstm_fallback(exp, f32in).astype(np.float32)

    expP = np.zeros((B, L_PAD, EMB), np.float32)
    expP[:, :L] = exp
    expR = expP[:, ::-1]

    in_maps = make_in_maps(expP, expR, inputs)

    if _COMPILED is None:
        _COMPILED = _build_kernel()
    nc = _COMPILED

    res = run_bass_kernel_spmd(nc, in_maps, core_ids=list(range(N_CORES)))

    out = np.empty((B, L_PAD, MEL), np.float32)
    for j in range(N_CORES):
        om = res.results[j]["out_mel"]          # [MEL, CHUNK, 2, B]
        blk = om.transpose(3, 2, 1, 0).reshape(B, CHUNK2, MEL)
        out[:, j * CHUNK2:(j + 1) * CHUNK2] = blk
    return np.ascontiguousarray(out[:, :L])


if __name__ == "__main__":
    inputs = dict(np.load("/root/problem/inputs.npz"))
    out = kernel(**inputs)
    ref = np.load("/root/problem/expected.npy")
    diff = np.abs(out - ref)
    print("out", out.shape, "absmax diff", diff.max(),
          "rel", diff.max() / np.abs(ref).max())


# revision 15
# speedup vs baseline: 1.1300x; 1.0662x over previous
"""MiniFastSpeech Trainium2 kernel (v6: 4-chain latency-hiding bf16 LSTM).

v3 (2 chains/core) measured loop-bound: the per-step recurrence
dependency chain (mms -> sigmoid -> DVE c-update -> tanh(c) -> h-write ->
mms) is ~4.5us while engine busy is only ~3.2us/step -- ~1.8us/step of
semaphore/pipeline dead time that scheduling cannot remove (every
DVE-produced value costs ~420ns to reach its consumer).

v6 goes busy-bound instead: 4 chains per core (2 fwd + 2 bwd, each 128
lanes = 2 seq-chunks x 64 batch; 32 chunks per direction, CHUNK=21,
W=12 warmup). The period must cover 4 chains' engine work (~6.4us on
Act) which exceeds the ~4.5us chain loop, so the recurrence latency
hides completely. Act work per chain-step: sigmoid [512] over bank A =
[i,f], tanh(x/2) [512] over bank B = [g,o] (g rows pre-doubled -> exact
tanh(g)), tanh [256] of c. DVE (bf16 2x): fc, ig, c_new tensor ops +
2 scalar_tensor_tensor h-writes computing h2 = (tanh(o/2)+1)*tanh(c) =
2h (whh/lin pre-halved on host absorb the 2x). PSUM: 8 banks = 4 chains
x 2 banks, bufs=1; xe matmuls run in-step (no prefetch; PE has slack).
Phase-2 final linear per chain-pair interleaved into the loop; bias add
on Pool.
"""

import sys
import numpy as np
from contextlib import ExitStack

sys.path.insert(0, "/opt/trn_rl_repo")

import concourse.bass as bass
import concourse.tile as tile
from concourse import bacc, mybir
from concourse.bass_utils import run_bass_kernel_spmd

# ---- problem constants (hardcoded per contract) ----
VOCAB, EMB, HID, MEL = 256, 128, 256, 80
B, T = 64, 512
N_CORES = 8
NCHUNK = 32          # chunks per direction
W = 12               # warmup steps per chain
CHUNK = 21           # positions per chunk; L_PAD = 672 >= L
L_PAD = NCHUNK * CHUNK
K_STEPS = W + CHUNK
F32 = mybir.dt.float32
BF16 = mybir.dt.bfloat16
SIG = mybir.ActivationFunctionType.Sigmoid
TANH = mybir.ActivationFunctionType.Tanh
MULT = mybir.AluOpType.mult
ADD = mybir.AluOpType.add

_COMPILED = None


def _host_expand(x, embed, dp_w, dp_b):
    xe = embed[x]                                   # (B,T,E)
    d = np.maximum(xe @ dp_w[0] + dp_b[0], 0)
    dur = np.floor(d).astype(np.int64) + 1
    cum = np.cumsum(dur, axis=1)
    L = int(cum[:, -1].max())
    pos = np.arange(L)
    idx = np.empty((B, L), np.int64)
    for b in range(B):
        idx[b] = np.searchsorted(cum[b], pos, side="right")
    mask = (pos[None, :] < cum[:, -1:]).astype(np.float32)
    exp = np.take_along_axis(xe, np.clip(idx, 0, T - 1)[..., None], axis=1)
    return np.ascontiguousarray(exp * mask[..., None], dtype=np.float32), L


# m-chunk order [i0 i1 f0 f1 | g0 g1 o0 o1]; rows in PyTorch [i,f,g,o] layout.
# Bank A = [i,f] -> one sigmoid act [512]. Bank B = [g,o] -> one tanh(x*0.5)
# act: g rows pre-doubled -> exact tanh(g) in cols 0:256; o gives tanh(o/2).
# whh: additionally all rows halved because the moving h operand is 2h.
def _mchunk_rows():
    rows, sc_ih, sc_hh = [], [], []
    for base, sc in ((0, 1.0), (HID, 1.0), (2 * HID, 2.0), (3 * HID, 1.0)):
        for half in (0, 1):
            rows.append(np.arange(base + half * 128, base + half * 128 + 128))
            sc_ih.append(np.full(128, sc, np.float32))
            sc_hh.append(np.full(128, sc * 0.5, np.float32))
    return (np.concatenate(rows), np.concatenate(sc_ih), np.concatenate(sc_hh))


class _Chain:
    def __init__(self, name, whh, wih, xe_cols, X, poolA, poolB):
        self.name = name
        self.whh = whh          # sbuf [128, 16*128] bf16, tile (m,k) at (2m+k)*128
        self.wih = wih          # sbuf [128, 8*128] bf16, tile m at m*128
        self.xe_cols = xe_cols  # slice in the xein tile
        self.X = X              # sbuf [128, 2*XW] bf16; k-half at k*XW
        self.poolA = poolA
        self.poolB = poolB
        self.gA = None
        self.gB = None
        self.src = None         # (h0, h1) col blocks [128,128] (2h of prev step)
        self.c_prev = None
        self.sf = None
        self.tB = None
        self.fc = None
        self.ig = None
        self.c_new = None
        self.tc = None


def _build_kernel():
    nc = bacc.Bacc("TRN2", target_bir_lowering=False, debug=False,
                   num_devices=N_CORES)

    xein = nc.dram_tensor("xein", [K_STEPS, EMB, 512], BF16,
                          kind="ExternalInput").ap()
    whh_f_d = nc.dram_tensor("whhT_f", [128, 16 * 128], BF16, kind="ExternalInput").ap()
    whh_b_d = nc.dram_tensor("whhT_b", [128, 16 * 128], BF16, kind="ExternalInput").ap()
    wih_f_d = nc.dram_tensor("wihT_f", [128, 8 * 128], BF16, kind="ExternalInput").ap()
    wih_b_d = nc.dram_tensor("wihT_b", [128, 8 * 128], BF16, kind="ExternalInput").ap()
    lin_w_d = nc.dram_tensor("linT", [128, 4 * MEL], BF16, kind="ExternalInput").ap()
    lin_b_d = nc.dram_tensor("lin_b", [MEL, 1], F32, kind="ExternalInput").ap()
    out_d = nc.dram_tensor("out_mel", [MEL, 2, CHUNK, 2, B], F32,
                           kind="ExternalOutput").ap()

    with tile.TileContext(nc) as tc, ExitStack() as ctx:
        wpool = ctx.enter_context(tc.tile_pool(name="weights", bufs=1))
        xpool = ctx.enter_context(tc.tile_pool(name="xstream", bufs=4))
        state = ctx.enter_context(tc.tile_pool(name="state", bufs=3))
        actp = ctx.enter_context(tc.tile_pool(name="acts", bufs=3))
        xbig = ctx.enter_context(tc.tile_pool(name="xbig", bufs=1))
        scr = ctx.enter_context(tc.tile_pool(name="scratch", bufs=3))
        psA = [ctx.enter_context(tc.tile_pool(name=f"gA{i}", bufs=1,
                                              space="PSUM")) for i in range(4)]
        psB = [ctx.enter_context(tc.tile_pool(name=f"gB{i}", bufs=1,
                                              space="PSUM")) for i in range(4)]
        ostage = ctx.enter_context(tc.tile_pool(name="ostage", bufs=2))

        # ---- memsets first (Pool queue) so the PE pre-warm starts at t~0
        hinit = wpool.tile([128, 256], BF16, tag="hinit")
        nc.gpsimd.memset(hinit[:], 0.0)
        zstat_bf = wpool.tile([128, 64], BF16, tag="zstatbf")
        nc.gpsimd.memset(zstat_bf[:], 0.0)

        # PE p-state pre-warm: burn the ramp on dummy matmuls while the
        # weight DMAs are in flight, so step 0 runs at full clock.
        warm = psB[3].tile([128, 512], F32, tag="g", name="pewarm")
        NWARM = 10
        for i in range(NWARM):
            nc.tensor.matmul(warm[0:64, 0:256], zstat_bf[:], hinit[:],
                             start=(i == 0), stop=(i == NWARM - 1))

        # ---- xe stream DMAs ----
        xe_tiles = {}

        def emit_dma(s):
            if s not in xe_tiles and s < K_STEPS:
                xe = xpool.tile([EMB, 512], BF16, tag="xe", name=f"xe{s}")
                nc.sync.dma_start(xe[:], xein[s])
                xe_tiles[s] = xe

        emit_dma(0)
        emit_dma(1)

        # ---- weights -> SBUF
        wih_f = wpool.tile([128, 8 * 128], BF16, tag="wihf")
        nc.scalar.dma_start(wih_f[:], wih_f_d[:])
        wih_b = wpool.tile([128, 8 * 128], BF16, tag="wihb")
        nc.gpsimd.dma_start(wih_b[:], wih_b_d[:])
        whh_f = wpool.tile([128, 16 * 128], BF16, tag="whhf")
        nc.sync.dma_start(whh_f[:], whh_f_d[:])
        whh_b = wpool.tile([128, 16 * 128], BF16, tag="whhb")
        nc.scalar.dma_start(whh_b[:], whh_b_d[:])
        lin_w = wpool.tile([128, 4 * MEL], BF16, tag="linw")
        nc.scalar.dma_start(lin_w[:], lin_w_d[:])
        lin_b = wpool.tile([MEL, 1], F32, tag="linb")
        nc.gpsimd.dma_start(lin_b[:], lin_b_d[:])

        XW = (CHUNK + 2) * 128
        Xs = [xbig.tile([128, 2 * XW], BF16, tag=f"X{i}", name=f"X{i}")
              for i in range(4)]

        # chains: FA, BA, FB, BB (emit order); xe col blocks FA 0:128,
        # FB 128:256, BA 256:384, BB 384:512
        chains = [
            _Chain("FA", whh_f, wih_f, slice(0, 128), Xs[0], psA[0], psB[0]),
            _Chain("BA", whh_b, wih_b, slice(256, 384), Xs[1], psA[1], psB[1]),
            _Chain("FB", whh_f, wih_f, slice(128, 256), Xs[2], psA[2], psB[2]),
            _Chain("BB", whh_b, wih_b, slice(384, 512), Xs[3], psA[3], psB[3]),
        ]
        for ch in chains:
            ch.src = (hinit[:, 0:128], hinit[:, 128:256])
            c0 = state.tile([128, 256], BF16, tag="c" + ch.name,
                            name=f"c0{ch.name}")
            nc.gpsimd.memset(c0[:], 0.0)
            ch.c_prev = c0

        def emit_mms(ch, s):
            # all of bank A (xe + rec, k-major), then bank B. bufs=1: the
            # tile from step s-1 is recycled; Tile waits on its readers.
            xe = xe_tiles[s]
            gA = ch.poolA.tile([128, 512], F32, tag="g", name=f"gA{ch.name}{s}")
            gB = ch.poolB.tile([128, 512], F32, tag="g", name=f"gB{ch.name}{s}")
            for g, mbase in ((gA, 0), (gB, 4)):
                for m in range(mbase, mbase + 4):
                    col = (m % 4) * 128
                    nc.tensor.matmul(g[:, col:col + 128],
                                     ch.wih[:, m * 128:(m + 1) * 128],
                                     xe[:, ch.xe_cols],
                                     start=(m == mbase),
                                     stop=(s == 0 and m == mbase + 3))
                if s > 0:
                    for k in (0, 1):
                        for m in range(mbase, mbase + 4):
                            last = (m == mbase + 3) and (k == 1)
                            col = (m % 4) * 128
                            nc.tensor.matmul(
                                g[:, col:col + 128],
                                ch.whh[:, (2 * m + k) * 128:(2 * m + k + 1) * 128],
                                ch.src[k],
                                start=False, stop=last)
            ch.gA, ch.gB = gA, gB

        # ---- phase 2 (final linear) groups per chain pair ----
        movs = {}
        for p, (Xf, Xb) in enumerate(((Xs[0], Xs[1]), (Xs[2], Xs[3]))):
            movs[p] = [X[:, k * XW:k * XW + CHUNK * 128].rearrange(
                           "p (t l) -> p t l", l=128)
                       for X in (Xf, Xb) for k in (0, 1)]
        gstate = {"gi": 0}

        def emit_group(p, p0, glen):
            n = glen * 128
            ps = psA[gstate["gi"] % 2].tile([MEL, 512], F32, tag="g",
                                            name=f"op{p}_{p0}")
            for k in range(4):
                nc.tensor.matmul(ps[:, 0:n], lin_w[:, k * MEL:(k + 1) * MEL],
                                 movs[p][k][:, p0:p0 + glen],
                                 start=(k == 0), stop=(k == 3))
            o_sb = ostage.tile([MEL, 512], F32, tag="os", name=f"os{p}_{p0}")
            nc.gpsimd.tensor_scalar(o_sb[:, 0:n], ps[:, 0:n], lin_b[:], None,
                                    ADD)
            q = (nc.sync, nc.gpsimd)[gstate["gi"] % 2]
            q.dma_start(out_d[:, p, p0:p0 + glen], o_sb[:, 0:n])
            gstate["gi"] += 1

        groups_at = {}
        for p in (0, 1):
            p0 = 0
            while p0 < CHUNK:
                glen = min(4, CHUNK - p0)
                ready = W + max(p0 + glen - 1, CHUNK - 1 - p0)
                groups_at.setdefault(min(ready + (p % 2), K_STEPS - 1),
                                     []).append((p, p0, glen))
                p0 += glen

        def emit_sf(ch, s):
            sf = actp.tile([128, 512], BF16, tag="sf" + ch.name,
                           name=f"sf{ch.name}{s}")
            nc.scalar.activation(sf[:], ch.gA[:], SIG)
            ch.sf = sf

        def emit_tB(ch, s):
            t = actp.tile([128, 512], BF16, tag="tB" + ch.name,
                          name=f"tB{ch.name}{s}")
            nc.scalar.activation(t[:], ch.gB[:], TANH, scale=0.5)
            ch.tB = t

        def emit_fc(ch, s):
            fc = scr.tile([128, 256], BF16, tag="fc" + ch.name,
                          name=f"fc{ch.name}{s}")
            nc.vector.tensor_mul(fc[:], ch.sf[:, 256:512], ch.c_prev[:])
            ch.fc = fc

        def emit_ig(ch, s):
            ig = scr.tile([128, 256], BF16, tag="ig" + ch.name,
                          name=f"ig{ch.name}{s}")
            nc.vector.tensor_mul(ig[:], ch.sf[:, 0:256], ch.tB[:, 0:256])
            ch.ig = ig

        def emit_cnew(ch, s):
            c_new = state.tile([128, 256], BF16, tag="c" + ch.name,
                               name=f"c{ch.name}{s}")
            nc.vector.tensor_add(c_new[:], ch.fc[:], ch.ig[:])
            ch.c_new = c_new

        def emit_tc(ch, s):
            tc_ = actp.tile([128, 256], BF16, tag="tc" + ch.name,
                            name=f"tc{ch.name}{s}")
            nc.scalar.activation(tc_[:], ch.c_new[:], TANH)
            ch.tc = tc_

        def emit_h(ch, s):
            real = s >= W
            t_rel = s - W
            if real:
                lp = t_rel if ch.name[0] == "F" else CHUNK - 1 - t_rel
            else:
                lp = CHUNK + (s & 1)
            dst = tuple(ch.X[:, k * XW + lp * 128:k * XW + (lp + 1) * 128]
                        for k in (0, 1))
            # h2 = (tanh(o/2) + 1) * tanh(c) = 2*sigma(o)*tanh(c); k-half
            # writes so next step's k0 recurrent mms start after half lands.
            for k in (0, 1):
                nc.vector.scalar_tensor_tensor(
                    dst[k], ch.tB[:, 256 + k * 128:256 + (k + 1) * 128], 1.0,
                    ch.tc[:, k * 128:(k + 1) * 128], ADD, MULT)
            ch.src = dst
            ch.c_prev = ch.c_new

        for s in range(K_STEPS):
            emit_dma(s + 1)
            for ch in chains:
                emit_mms(ch, s)
            emit_dma(s + 2)

            # Act queue: sfFA tBFA sfBA tBBA tcFA sfFB tBFB tcBA sfBB tBBB
            # tcFB tcBB; DVE trails each chain's acts.
            c0, c1, c2, c3 = chains
            emit_sf(c0, s)
            emit_tB(c0, s)
            emit_sf(c1, s)
            emit_fc(c0, s)
            emit_ig(c0, s)
            emit_cnew(c0, s)
            emit_tB(c1, s)
            emit_tc(c0, s)
            emit_fc(c1, s)
            emit_ig(c1, s)
            emit_cnew(c1, s)
            emit_sf(c2, s)
            emit_h(c0, s)
            emit_tB(c2, s)
            emit_tc(c1, s)
            emit_fc(c2, s)
            emit_ig(c2, s)
            emit_cnew(c2, s)
            emit_h(c1, s)
            emit_sf(c3, s)
            emit_tB(c3, s)
            emit_tc(c2, s)
            emit_fc(c3, s)
            emit_ig(c3, s)
            emit_cnew(c3, s)
            emit_h(c2, s)
            emit_tc(c3, s)
            emit_h(c3, s)

            for (p, p0g, gl) in groups_at.get(s, []):
                emit_group(p, p0g, gl)

    nc.compile()
    return nc


def _np_lstm_fallback(exp, inputs):
    def sigmoid(z):
        return 1.0 / (1.0 + np.exp(-z))

    def lstm(xs, wih, whh, bih, bhh):
        Bb, L, E = xs.shape
        pre = np.einsum("ble,ge->blg", xs, wih) + bih + bhh
        h = np.zeros((Bb, HID), np.float32)
        c = np.zeros((Bb, HID), np.float32)
        hs = np.zeros((Bb, L, HID), np.float32)
        for t in range(L):
            gg = pre[:, t] + h @ whh.T
            i, f, g_, o = np.split(gg, 4, axis=-1)
            c = sigmoid(f) * c + sigmoid(i) * np.tanh(g_)
            h = sigmoid(o) * np.tanh(c)
            hs[:, t] = h
        return hs

    out_f = lstm(exp, inputs["wih_f"], inputs["whh_f"], inputs["bih_f"],
                 inputs["bhh_f"])
    out_b = lstm(exp[:, ::-1], inputs["wih_b"], inputs["whh_b"],
                 inputs["bih_b"], inputs["bhh_b"])[:, ::-1]
    out = np.concatenate([out_f, out_b], axis=-1)
    return out @ inputs["lin_w"].T + inputs["lin_b"]


def make_in_maps(expP, expR, inputs):
    import ml_dtypes
    bf16 = ml_dtypes.bfloat16
    rows, sc_ih, sc_hh = _mchunk_rows()

    def stat_tiles(w, scale):
        wp = (w.astype(np.float32)[rows] * scale[:, None])
        nk = w.shape[1] // 128
        out = np.zeros((128, 8 * nk * 128), np.float32)
        for m in range(8):
            for k in range(nk):
                out[:, (m * nk + k) * 128:(m * nk + k + 1) * 128] = \
                    wp[m * 128:(m + 1) * 128, k * 128:(k + 1) * 128].T
        return np.ascontiguousarray(out).astype(bf16)

    whhT_f = stat_tiles(inputs["whh_f"], sc_hh)
    whhT_b = stat_tiles(inputs["whh_b"], sc_hh)
    wihT_f = stat_tiles(inputs["wih_f"], sc_ih)
    wihT_b = stat_tiles(inputs["wih_b"], sc_ih)
    lw = inputs["lin_w"].astype(np.float32) * 0.5
    linT = np.concatenate([np.ascontiguousarray(lw[:, k * 128:(k + 1) * 128].T)
                           for k in range(4)], axis=1).astype(bf16)
    lin_b2 = np.ascontiguousarray(inputs["lin_b"].astype(np.float32)[:, None])

    in_maps = []
    for j in range(N_CORES):
        xein = np.zeros((K_STEPS, EMB, 512), np.float32)
        # lane blocks of 64: FA=(4j,4j+1), FB=(4j+2,4j+3),
        # BA=(31-4j,30-4j), BB=(29-4j,28-4j); xe cols FA 0:128, FB 128:256,
        # BA 256:384, BB 384:512
        cks = [4 * j, 4 * j + 1, 4 * j + 2, 4 * j + 3,
               31 - 4 * j, 30 - 4 * j, 29 - 4 * j, 28 - 4 * j]
        srcs = [expP] * 4 + [expR] * 4
        for s in range(K_STEPS):
            for ci, (ck, src) in enumerate(zip(cks, srcs)):
                p = ck * CHUNK - W + s
                if 0 <= p < L_PAD:
                    xein[s, :, ci * 64:(ci + 1) * 64] = src[:, p].T
        in_maps.append({
            "xein": xein.astype(bf16),
            "whhT_f": whhT_f, "whhT_b": whhT_b,
            "wihT_f": wihT_f, "wihT_b": wihT_b,
            "linT": linT, "lin_b": lin_b2,
        })
    return in_maps


def kernel(**inputs):
    global _COMPILED
    inputs = {k: np.asarray(v) for k, v in inputs.items()}
    x = inputs["x"].astype(np.int64)
    exp, L = _host_expand(x, inputs["embed"].astype(np.float32),
                          inputs["dp_w"].astype(np.float32),
                          inputs["dp_b"].astype(np.float32))

    bias_mag = max(float(np.abs(inputs[k]).max())
                   for k in ("bih_f", "bhh_f", "bih_b", "bhh_b"))
    if L > L_PAD or bias_mag != 0.0:
        f32in = {k: (v.astype(np.float32) if v.dtype.kind == "f" else v)
                 for k, v in inputs.items()}
        return _np_lstm_fallback(exp, f32in).astype(np.float32)

    expP = np.zeros((B, L_PAD, EMB), np.float32)
    expP[:, :L] = exp
    expR = expP[:, ::-1]

    in_maps = make_in_maps(expP, expR, inputs)

    if _COMPILED is None:
        _COMPILED = _build_kernel()
    nc = _COMPILED

    res = run_bass_kernel_spmd(nc, in_maps, core_ids=list(range(N_CORES)))

    out = np.empty((B, L_PAD, MEL), np.float32)
    for j in range(N_CORES):
        om = res.results[j]["out_mel"]          # [MEL, 2, CHUNK, 2, B]
        for p in (0, 1):
            for half in (0, 1):
                seg = om[:, p, :, half, :]      # [MEL, CHUNK, B]
                c = 4 * j + 2 * p + half
                out[:, c * CHUNK:(c + 1) * CHUNK] = seg.transpose(2, 1, 0)
    return np.ascontiguousarray(out[:, :L])


if __name__ == "__main__":
    inputs = dict(np.load("/root/problem/inputs.npz"))
    out = kernel(**inputs)
    ref = np.load("/root/problem/expected.npy")
    diff = np.abs(out - ref)
    print("out", out.shape, "absmax diff", diff.max(),
          "rel", diff.max() / np.abs(ref).max())


# revision 16
# speedup vs baseline: 1.1306x; 1.0005x over previous
"""MiniFastSpeech Trainium2 kernel (v6: 4-chain latency-hiding bf16 LSTM).

v3 (2 chains/core) measured loop-bound: the per-step recurrence
dependency chain (mms -> sigmoid -> DVE c-update -> tanh(c) -> h-write ->
mms) is ~4.5us while engine busy is only ~3.2us/step -- ~1.8us/step of
semaphore/pipeline dead time that scheduling cannot remove (every
DVE-produced value costs ~420ns to reach its consumer).

v6 goes busy-bound instead: 4 chains per core (2 fwd + 2 bwd, each 128
lanes = 2 seq-chunks x 64 batch; 32 chunks per direction, CHUNK=21,
W=12 warmup). The period must cover 4 chains' engine work (~6.4us on
Act) which exceeds the ~4.5us chain loop, so the recurrence latency
hides completely. Act work per chain-step: sigmoid [512] over bank A =
[i,f], tanh(x/2) [512] over bank B = [g,o] (g rows pre-doubled -> exact
tanh(g)), tanh [256] of c. DVE (bf16 2x): fc, ig, c_new tensor ops +
2 scalar_tensor_tensor h-writes computing h2 = (tanh(o/2)+1)*tanh(c) =
2h (whh/lin pre-halved on host absorb the 2x). PSUM: 8 banks = 4 chains
x 2 banks, bufs=1; xe matmuls run in-step (no prefetch; PE has slack).
Phase-2 final linear per chain-pair interleaved into the loop; bias add
on Pool.
"""

import sys
import numpy as np
from contextlib import ExitStack

sys.path.insert(0, "/opt/trn_rl_repo")

import concourse.bass as bass
import concourse.tile as tile
from concourse import bacc, mybir
from concourse.bass_utils import run_bass_kernel_spmd

# ---- problem constants (hardcoded per contract) ----
VOCAB, EMB, HID, MEL = 256, 128, 256, 80
B, T = 64, 512
N_CORES = 8
NCHUNK = 32          # chunks per direction
W = 12               # warmup steps per chain
CHUNK = 21           # positions per chunk; L_PAD = 672 >= L
L_PAD = NCHUNK * CHUNK
K_STEPS = W + CHUNK
F32 = mybir.dt.float32
BF16 = mybir.dt.bfloat16
SIG = mybir.ActivationFunctionType.Sigmoid
TANH = mybir.ActivationFunctionType.Tanh
MULT = mybir.AluOpType.mult
ADD = mybir.AluOpType.add

_COMPILED = None


def _host_expand(x, embed, dp_w, dp_b):
    xe = embed[x]                                   # (B,T,E)
    d = np.maximum(xe @ dp_w[0] + dp_b[0], 0)
    dur = np.floor(d).astype(np.int64) + 1
    cum = np.cumsum(dur, axis=1)
    L = int(cum[:, -1].max())
    pos = np.arange(L)
    idx = np.empty((B, L), np.int64)
    for b in range(B):
        idx[b] = np.searchsorted(cum[b], pos, side="right")
    mask = (pos[None, :] < cum[:, -1:]).astype(np.float32)
    exp = np.take_along_axis(xe, np.clip(idx, 0, T - 1)[..., None], axis=1)
    return np.ascontiguousarray(exp * mask[..., None], dtype=np.float32), L


# m-chunk order [i0 i1 f0 f1 | g0 g1 o0 o1]; rows in PyTorch [i,f,g,o] layout.
# Bank A = [i,f] -> one sigmoid act [512]. Bank B = [g,o] -> one tanh(x*0.5)
# act: g rows pre-doubled -> exact tanh(g) in cols 0:256; o gives tanh(o/2).
# whh: additionally all rows halved because the moving h operand is 2h.
def _mchunk_rows():
    rows, sc_ih, sc_hh = [], [], []
    for base, sc in ((0, 1.0), (HID, 1.0), (2 * HID, 2.0), (3 * HID, 1.0)):
        for half in (0, 1):
            rows.append(np.arange(base + half * 128, base + half * 128 + 128))
            sc_ih.append(np.full(128, sc, np.float32))
            sc_hh.append(np.full(128, sc * 0.5, np.float32))
    return (np.concatenate(rows), np.concatenate(sc_ih), np.concatenate(sc_hh))


class _Chain:
    def __init__(self, name, whh, wih, xe_cols, X, poolA, poolB):
        self.name = name
        self.whh = whh          # sbuf [128, 16*128] bf16, tile (m,k) at (2m+k)*128
        self.wih = wih          # sbuf [128, 8*128] bf16, tile m at m*128
        self.xe_cols = xe_cols  # slice in the xein tile
        self.X = X              # sbuf [128, 2*XW] bf16; k-half at k*XW
        self.poolA = poolA
        self.poolB = poolB
        self.gA = None
        self.gB = None
        self.src = None         # (h0, h1) col blocks [128,128] (2h of prev step)
        self.c_prev = None
        self.sf = None
        self.tB = None
        self.fc = None
        self.ig = None
        self.c_new = None
        self.tc = None


def _build_kernel():
    nc = bacc.Bacc("TRN2", target_bir_lowering=False, debug=False,
                   num_devices=N_CORES)

    xein = nc.dram_tensor("xein", [K_STEPS, EMB, 512], BF16,
                          kind="ExternalInput").ap()
    whh_f_d = nc.dram_tensor("whhT_f", [128, 16 * 128], BF16, kind="ExternalInput").ap()
    whh_b_d = nc.dram_tensor("whhT_b", [128, 16 * 128], BF16, kind="ExternalInput").ap()
    wih_f_d = nc.dram_tensor("wihT_f", [128, 8 * 128], BF16, kind="ExternalInput").ap()
    wih_b_d = nc.dram_tensor("wihT_b", [128, 8 * 128], BF16, kind="ExternalInput").ap()
    lin_w_d = nc.dram_tensor("linT", [128, 4 * MEL], BF16, kind="ExternalInput").ap()
    lin_b_d = nc.dram_tensor("lin_b", [MEL, 1], F32, kind="ExternalInput").ap()
    out_d = nc.dram_tensor("out_mel", [MEL, 2, CHUNK, 2, B], F32,
                           kind="ExternalOutput").ap()

    with tile.TileContext(nc) as tc, ExitStack() as ctx:
        wpool = ctx.enter_context(tc.tile_pool(name="weights", bufs=1))
        xpool = ctx.enter_context(tc.tile_pool(name="xstream", bufs=4))
        state = ctx.enter_context(tc.tile_pool(name="state", bufs=3))
        actp = ctx.enter_context(tc.tile_pool(name="acts", bufs=3))
        xbig = ctx.enter_context(tc.tile_pool(name="xbig", bufs=1))
        scr = ctx.enter_context(tc.tile_pool(name="scratch", bufs=3))
        psA = [ctx.enter_context(tc.tile_pool(name=f"gA{i}", bufs=1,
                                              space="PSUM")) for i in range(4)]
        psB = [ctx.enter_context(tc.tile_pool(name=f"gB{i}", bufs=1,
                                              space="PSUM")) for i in range(4)]
        ostage = ctx.enter_context(tc.tile_pool(name="ostage", bufs=2))

        # ---- memsets first (Pool queue) so the PE pre-warm starts at t~0
        hinit = wpool.tile([128, 256], BF16, tag="hinit")
        nc.gpsimd.memset(hinit[:], 0.0)
        zstat_bf = wpool.tile([128, 64], BF16, tag="zstatbf")
        nc.gpsimd.memset(zstat_bf[:], 0.0)

        # PE p-state pre-warm: burn the ramp on dummy matmuls while the
        # weight DMAs are in flight, so step 0 runs at full clock.
        warm = psB[3].tile([128, 512], F32, tag="g", name="pewarm")
        NWARM = 10
        for i in range(NWARM):
            nc.tensor.matmul(warm[0:64, 0:256], zstat_bf[:], hinit[:],
                             start=(i == 0), stop=(i == NWARM - 1))

        # ---- xe stream DMAs ----
        xe_tiles = {}

        def emit_dma(s):
            if s not in xe_tiles and s < K_STEPS:
                xe = xpool.tile([EMB, 512], BF16, tag="xe", name=f"xe{s}")
                nc.sync.dma_start(xe[:], xein[s])
                xe_tiles[s] = xe

        emit_dma(0)
        emit_dma(1)

        # ---- weights -> SBUF
        wih_f = wpool.tile([128, 8 * 128], BF16, tag="wihf")
        nc.scalar.dma_start(wih_f[:], wih_f_d[:])
        wih_b = wpool.tile([128, 8 * 128], BF16, tag="wihb")
        nc.gpsimd.dma_start(wih_b[:], wih_b_d[:])
        whh_f = wpool.tile([128, 16 * 128], BF16, tag="whhf")
        nc.sync.dma_start(whh_f[:], whh_f_d[:])
        whh_b = wpool.tile([128, 16 * 128], BF16, tag="whhb")
        nc.scalar.dma_start(whh_b[:], whh_b_d[:])
        lin_w = wpool.tile([128, 4 * MEL], BF16, tag="linw")
        nc.scalar.dma_start(lin_w[:], lin_w_d[:])
        lin_b = wpool.tile([MEL, 1], F32, tag="linb")
        nc.gpsimd.dma_start(lin_b[:], lin_b_d[:])

        XW = (CHUNK + 2) * 128
        Xs = [xbig.tile([128, 2 * XW], BF16, tag=f"X{i}", name=f"X{i}")
              for i in range(4)]

        # chains: FA, BA, FB, BB (emit order); xe col blocks FA 0:128,
        # FB 128:256, BA 256:384, BB 384:512
        chains = [
            _Chain("FA", whh_f, wih_f, slice(0, 128), Xs[0], psA[0], psB[0]),
            _Chain("BA", whh_b, wih_b, slice(256, 384), Xs[1], psA[1], psB[1]),
            _Chain("FB", whh_f, wih_f, slice(128, 256), Xs[2], psA[2], psB[2]),
            _Chain("BB", whh_b, wih_b, slice(384, 512), Xs[3], psA[3], psB[3]),
        ]
        for ch in chains:
            ch.src = (hinit[:, 0:128], hinit[:, 128:256])
            c0 = state.tile([128, 256], BF16, tag="c" + ch.name,
                            name=f"c0{ch.name}")
            nc.gpsimd.memset(c0[:], 0.0)
            ch.c_prev = c0

        def emit_mms(ch, s):
            # all of bank A (xe + rec, k-major), then bank B. bufs=1: the
            # tile from step s-1 is recycled; Tile waits on its readers.
            xe = xe_tiles[s]
            gA = ch.poolA.tile([128, 512], F32, tag="g", name=f"gA{ch.name}{s}")
            gB = ch.poolB.tile([128, 512], F32, tag="g", name=f"gB{ch.name}{s}")
            for g, mbase in ((gA, 0), (gB, 4)):
                for m in range(mbase, mbase + 4):
                    col = (m % 4) * 128
                    nc.tensor.matmul(g[:, col:col + 128],
                                     ch.wih[:, m * 128:(m + 1) * 128],
                                     xe[:, ch.xe_cols],
                                     start=(m == mbase),
                                     stop=(s == 0 and m == mbase + 3))
                if s > 0:
                    for k in (0, 1):
                        for m in range(mbase, mbase + 4):
                            last = (m == mbase + 3) and (k == 1)
                            col = (m % 4) * 128
                            nc.tensor.matmul(
                                g[:, col:col + 128],
                                ch.whh[:, (2 * m + k) * 128:(2 * m + k + 1) * 128],
                                ch.src[k],
                                start=False, stop=last)
            ch.gA, ch.gB = gA, gB

        # ---- phase 2 (final linear) groups per chain pair ----
        movs = {}
        for p, (Xf, Xb) in enumerate(((Xs[0], Xs[1]), (Xs[2], Xs[3]))):
            movs[p] = [X[:, k * XW:k * XW + CHUNK * 128].rearrange(
                           "p (t l) -> p t l", l=128)
                       for X in (Xf, Xb) for k in (0, 1)]
        gstate = {"gi": 0}

        def emit_group(p, p0, glen):
            n = glen * 128
            ps = psA[gstate["gi"] % 2].tile([MEL, 512], F32, tag="g",
                                            name=f"op{p}_{p0}")
            for k in range(4):
                nc.tensor.matmul(ps[:, 0:n], lin_w[:, k * MEL:(k + 1) * MEL],
                                 movs[p][k][:, p0:p0 + glen],
                                 start=(k == 0), stop=(k == 3))
            o_sb = ostage.tile([MEL, 512], F32, tag="os", name=f"os{p}_{p0}")
            nc.vector.tensor_scalar(o_sb[:, 0:n], ps[:, 0:n], lin_b[:], None,
                                    ADD)
            q = (nc.sync, nc.gpsimd)[gstate["gi"] % 2]
            q.dma_start(out_d[:, p, p0:p0 + glen], o_sb[:, 0:n])
            gstate["gi"] += 1

        groups_at = {}
        for p in (0, 1):
            p0 = 0
            while p0 < CHUNK:
                glen = min(4, CHUNK - p0)
                ready = W + max(p0 + glen - 1, CHUNK - 1 - p0)
                groups_at.setdefault(min(ready + (p % 2), K_STEPS - 1),
                                     []).append((p, p0, glen))
                p0 += glen

        def emit_sf(ch, s):
            sf = actp.tile([128, 512], BF16, tag="sf" + ch.name,
                           name=f"sf{ch.name}{s}")
            nc.scalar.activation(sf[:], ch.gA[:], SIG)
            ch.sf = sf

        def emit_tB(ch, s):
            t = actp.tile([128, 512], BF16, tag="tB" + ch.name,
                          name=f"tB{ch.name}{s}")
            nc.scalar.activation(t[:], ch.gB[:], TANH, scale=0.5)
            ch.tB = t

        def emit_fc(ch, s):
            fc = scr.tile([128, 256], BF16, tag="fc" + ch.name,
                          name=f"fc{ch.name}{s}")
            nc.vector.tensor_mul(fc[:], ch.sf[:, 256:512], ch.c_prev[:])
            ch.fc = fc

        def emit_ig(ch, s):
            ig = scr.tile([128, 256], BF16, tag="ig" + ch.name,
                          name=f"ig{ch.name}{s}")
            nc.vector.tensor_mul(ig[:], ch.sf[:, 0:256], ch.tB[:, 0:256])
            ch.ig = ig

        def emit_cnew(ch, s):
            c_new = state.tile([128, 256], BF16, tag="c" + ch.name,
                               name=f"c{ch.name}{s}")
            nc.vector.tensor_add(c_new[:], ch.fc[:], ch.ig[:])
            ch.c_new = c_new

        def emit_tc(ch, s):
            tc_ = actp.tile([128, 256], BF16, tag="tc" + ch.name,
                            name=f"tc{ch.name}{s}")
            nc.scalar.activation(tc_[:], ch.c_new[:], TANH)
            ch.tc = tc_

        def emit_h(ch, s):
            real = s >= W
            t_rel = s - W
            if real:
                lp = t_rel if ch.name[0] == "F" else CHUNK - 1 - t_rel
            else:
                lp = CHUNK + (s & 1)
            dst = tuple(ch.X[:, k * XW + lp * 128:k * XW + (lp + 1) * 128]
                        for k in (0, 1))
            # h2 = (tanh(o/2) + 1) * tanh(c) = 2*sigma(o)*tanh(c); k-half
            # writes so next step's k0 recurrent mms start after half lands.
            for k in (0, 1):
                nc.vector.scalar_tensor_tensor(
                    dst[k], ch.tB[:, 256 + k * 128:256 + (k + 1) * 128], 1.0,
                    ch.tc[:, k * 128:(k + 1) * 128], ADD, MULT)
            ch.src = dst
            ch.c_prev = ch.c_new

        for s in range(K_STEPS):
            emit_dma(s + 1)
            for ch in chains:
                emit_mms(ch, s)
            emit_dma(s + 2)

            # Act queue: sfFA tBFA sfBA tBBA tcFA sfFB tBFB tcBA sfBB tBBB
            # tcFB tcBB; DVE trails each chain's acts.
            c0, c1, c2, c3 = chains
            emit_sf(c0, s)
            emit_tB(c0, s)
            emit_sf(c1, s)
            emit_fc(c0, s)
            emit_ig(c0, s)
            emit_cnew(c0, s)
            emit_tB(c1, s)
            emit_tc(c0, s)
            emit_fc(c1, s)
            emit_ig(c1, s)
            emit_cnew(c1, s)
            emit_sf(c2, s)
            emit_h(c0, s)
            emit_tB(c2, s)
            emit_tc(c1, s)
            emit_fc(c2, s)
            emit_ig(c2, s)
            emit_cnew(c2, s)
            emit_h(c1, s)
            emit_sf(c3, s)
            emit_tB(c3, s)
            emit_tc(c2, s)
            emit_fc(c3, s)
            emit_ig(c3, s)
            emit_cnew(c3, s)
            emit_h(c2, s)
            emit_tc(c3, s)
            emit_h(c3, s)

            for (p, p0g, gl) in groups_at.get(s, []):
                emit_group(p, p0g, gl)

    nc.compile()
    return nc


def _np_lstm_fallback(exp, inputs):
    def sigmoid(z):
        return 1.0 / (1.0 + np.exp(-z))

    def lstm(xs, wih, whh, bih, bhh):
        Bb, L, E = xs.shape
        pre = np.einsum("ble,ge->blg", xs, wih) + bih + bhh
        h = np.zeros((Bb, HID), np.float32)
        c = np.zeros((Bb, HID), np.float32)
        hs = np.zeros((Bb, L, HID), np.float32)
        for t in range(L):
            gg = pre[:, t] + h @ whh.T
            i, f, g_, o = np.split(gg, 4, axis=-1)
            c = sigmoid(f) * c + sigmoid(i) * np.tanh(g_)
            h = sigmoid(o) * np.tanh(c)
            hs[:, t] = h
        return hs

    out_f = lstm(exp, inputs["wih_f"], inputs["whh_f"], inputs["bih_f"],
                 inputs["bhh_f"])
    out_b = lstm(exp[:, ::-1], inputs["wih_b"], inputs["whh_b"],
                 inputs["bih_b"], inputs["bhh_b"])[:, ::-1]
    out = np.concatenate([out_f, out_b], axis=-1)
    return out @ inputs["lin_w"].T + inputs["lin_b"]


def make_in_maps(expP, expR, inputs):
    import ml_dtypes
    bf16 = ml_dtypes.bfloat16
    rows, sc_ih, sc_hh = _mchunk_rows()

    def stat_tiles(w, scale):
        wp = (w.astype(np.float32)[rows] * scale[:, None])
        nk = w.shape[1] // 128
        out = np.zeros((128, 8 * nk * 128), np.float32)
        for m in range(8):
            for k in range(nk):
                out[:, (m * nk + k) * 128:(m * nk + k + 1) * 128] = \
                    wp[m * 128:(m + 1) * 128, k * 128:(k + 1) * 128].T
        return np.ascontiguousarray(out).astype(bf16)

    whhT_f = stat_tiles(inputs["whh_f"], sc_hh)
    whhT_b = stat_tiles(inputs["whh_b"], sc_hh)
    wihT_f = stat_tiles(inputs["wih_f"], sc_ih)
    wihT_b = stat_tiles(inputs["wih_b"], sc_ih)
    lw = inputs["lin_w"].astype(np.float32) * 0.5
    linT = np.concatenate([np.ascontiguousarray(lw[:, k * 128:(k + 1) * 128].T)
                           for k in range(4)], axis=1).astype(bf16)
    lin_b2 = np.ascontiguousarray(inputs["lin_b"].astype(np.float32)[:, None])

    in_maps = []
    for j in range(N_CORES):
        xein = np.zeros((K_STEPS, EMB, 512), np.float32)
        # lane blocks of 64: FA=(4j,4j+1), FB=(4j+2,4j+3),
        # BA=(31-4j,30-4j), BB=(29-4j,28-4j); xe cols FA 0:128, FB 128:256,
        # BA 256:384, BB 384:512
        cks = [4 * j, 4 * j + 1, 4 * j + 2, 4 * j + 3,
               31 - 4 * j, 30 - 4 * j, 29 - 4 * j, 28 - 4 * j]
        srcs = [expP] * 4 + [expR] * 4
        for s in range(K_STEPS):
            for ci, (ck, src) in enumerate(zip(cks, srcs)):
                p = ck * CHUNK - W + s
                if 0 <= p < L_PAD:
                    xein[s, :, ci * 64:(ci + 1) * 64] = src[:, p].T
        in_maps.append({
            "xein": xein.astype(bf16),
            "whhT_f": whhT_f, "whhT_b": whhT_b,
            "wihT_f": wihT_f, "wihT_b": wihT_b,
            "linT": linT, "lin_b": lin_b2,
        })
    return in_maps


def kernel(**inputs):
    global _COMPILED
    inputs = {k: np.asarray(v) for k, v in inputs.items()}
    x = inputs["x"].astype(np.int64)
    exp, L = _host_expand(x, inputs["embed"].astype(np.float32),
                          inputs["dp_w"].astype(np.float32),
                          inputs["dp_b"].astype(np.float32))

    bias_mag = max(float(np.abs(inputs[k]).max())
                   for k in ("bih_f", "bhh_f", "bih_b", "bhh_b"))
    if L > L_PAD or bias_mag != 0.0:
        f32in = {k: (v.astype(np.float32) if v.dtype.kind == "f" else v)
                 for k, v in inputs.items()}
        return _np_lstm_fallback(exp, f32in).astype(np.float32)

    expP = np.zeros((B, L_PAD, EMB), np.float32)
    expP[:, :L] = exp
    expR = expP[:, ::-1]

    in_maps = make_in_maps(expP, expR, inputs)

    if _COMPILED is None:
        _COMPILED = _build_kernel()
    nc = _COMPILED

    res = run_bass_kernel_spmd(nc, in_maps, core_ids=list(range(N_CORES)))

    out = np.empty((B, L_PAD, MEL), np.float32)
    for j in range(N_CORES):
        om = res.results[j]["out_mel"]          # [MEL, 2, CHUNK, 2, B]
        for p in (0, 1):
            for half in (0, 1):
                seg = om[:, p, :, half, :]      # [MEL, CHUNK, B]
                c = 4 * j + 2 * p + half
                out[:, c * CHUNK:(c + 1) * CHUNK] = seg.transpose(2, 1, 0)
    return np.ascontiguousarray(out[:, :L])


if __name__ == "__main__":
    inputs = dict(np.load("/root/problem/inputs.npz"))
    out = kernel(**inputs)
    ref = np.load("/root/problem/expected.npy")
    diff = np.abs(out - ref)
    print("out", out.shape, "absmax diff", diff.max(),
          "rel", diff.max() / np.abs(ref).max())


# revision 19
# speedup vs baseline: 1.1983x; 1.0599x over previous
"""MiniFastSpeech Trainium2 kernel (v6: 4-chain latency-hiding bf16 LSTM).

v3 (2 chains/core) measured loop-bound: the per-step recurrence
dependency chain (mms -> sigmoid -> DVE c-update -> tanh(c) -> h-write ->
mms) is ~4.5us while engine busy is only ~3.2us/step -- ~1.8us/step of
semaphore/pipeline dead time that scheduling cannot remove (every
DVE-produced value costs ~420ns to reach its consumer).

v6 goes busy-bound instead: 4 chains per core (2 fwd + 2 bwd, each 128
lanes = 2 seq-chunks x 64 batch; 32 chunks per direction, CHUNK=21,
W=12 warmup). The period must cover 4 chains' engine work (~6.4us on
Act) which exceeds the ~4.5us chain loop, so the recurrence latency
hides completely. Act work per chain-step: sigmoid [512] over bank A =
[i,f], tanh(x/2) [512] over bank B = [g,o] (g rows pre-doubled -> exact
tanh(g)), tanh [256] of c. DVE (bf16 2x): fc, ig, c_new tensor ops +
2 scalar_tensor_tensor h-writes computing h2 = (tanh(o/2)+1)*tanh(c) =
2h (whh/lin pre-halved on host absorb the 2x). PSUM: 8 banks = 4 chains
x 2 banks, bufs=1; xe matmuls run in-step (no prefetch; PE has slack).
Phase-2 final linear per chain-pair interleaved into the loop; bias add
on Pool.
"""

import sys
import numpy as np
from contextlib import ExitStack

sys.path.insert(0, "/opt/trn_rl_repo")

import concourse.bass as bass
import concourse.tile as tile
from concourse import bacc, mybir
from concourse.bass_utils import run_bass_kernel_spmd

# ---- problem constants (hardcoded per contract) ----
VOCAB, EMB, HID, MEL = 256, 128, 256, 80
B, T = 64, 512
N_CORES = 8
NCHUNK = 32          # chunks per direction
W = 11               # warmup steps per chain
CHUNK = 21           # positions per chunk; L_PAD = 672 >= L
L_PAD = NCHUNK * CHUNK
K_STEPS = W + CHUNK
F32 = mybir.dt.float32
BF16 = mybir.dt.bfloat16
SIG = mybir.ActivationFunctionType.Sigmoid
TANH = mybir.ActivationFunctionType.Tanh
MULT = mybir.AluOpType.mult
ADD = mybir.AluOpType.add

_COMPILED = None


def _host_expand(x, embed, dp_w, dp_b):
    xe = embed[x]                                   # (B,T,E)
    d = np.maximum(xe @ dp_w[0] + dp_b[0], 0)
    dur = np.floor(d).astype(np.int64) + 1
    cum = np.cumsum(dur, axis=1)
    L = int(cum[:, -1].max())
    pos = np.arange(L)
    idx = np.empty((B, L), np.int64)
    for b in range(B):
        idx[b] = np.searchsorted(cum[b], pos, side="right")
    mask = (pos[None, :] < cum[:, -1:]).astype(np.float32)
    exp = np.take_along_axis(xe, np.clip(idx, 0, T - 1)[..., None], axis=1)
    return np.ascontiguousarray(exp * mask[..., None], dtype=np.float32), L


# m-chunk order [i0 i1 f0 f1 | g0 g1 o0 o1]; rows in PyTorch [i,f,g,o] layout.
# Bank A = [i,f] -> one sigmoid act [512]. Bank B = [g,o] -> one tanh(x*0.5)
# act: g rows pre-doubled -> exact tanh(g) in cols 0:256; o gives tanh(o/2).
# whh: additionally all rows halved because the moving h operand is 2h.
def _mchunk_rows():
    rows, sc_ih, sc_hh = [], [], []
    for base, sc in ((0, 1.0), (HID, 1.0), (2 * HID, 2.0), (3 * HID, 1.0)):
        for half in (0, 1):
            rows.append(np.arange(base + half * 128, base + half * 128 + 128))
            sc_ih.append(np.full(128, sc, np.float32))
            sc_hh.append(np.full(128, sc * 0.5, np.float32))
    return (np.concatenate(rows), np.concatenate(sc_ih), np.concatenate(sc_hh))


class _Chain:
    def __init__(self, name, whh, wih, xe_cols, X, poolA, poolB):
        self.name = name
        self.whh = whh          # sbuf [128, 16*128] bf16, tile (m,k) at (2m+k)*128
        self.wih = wih          # sbuf [128, 8*128] bf16, tile m at m*128
        self.xe_cols = xe_cols  # slice in the xein tile
        self.X = X              # sbuf [128, 2*XW] bf16; k-half at k*XW
        self.poolA = poolA
        self.poolB = poolB
        self.gA = None
        self.gB = None
        self.src = None         # (h0, h1) col blocks [128,128] (2h of prev step)
        self.c_prev = None
        self.sf = None
        self.tB = None
        self.fc = None
        self.ig = None
        self.c_new = None
        self.tc = None


def _build_kernel():
    nc = bacc.Bacc("TRN2", target_bir_lowering=False, debug=False,
                   num_devices=N_CORES)

    xein = nc.dram_tensor("xein", [K_STEPS, EMB, 512], BF16,
                          kind="ExternalInput").ap()
    whh_f_d = nc.dram_tensor("whhT_f", [128, 16 * 128], BF16, kind="ExternalInput").ap()
    whh_b_d = nc.dram_tensor("whhT_b", [128, 16 * 128], BF16, kind="ExternalInput").ap()
    wih_f_d = nc.dram_tensor("wihT_f", [128, 8 * 128], BF16, kind="ExternalInput").ap()
    wih_b_d = nc.dram_tensor("wihT_b", [128, 8 * 128], BF16, kind="ExternalInput").ap()
    lin_w_d = nc.dram_tensor("linT", [128, 4 * MEL], BF16, kind="ExternalInput").ap()
    lin_b_d = nc.dram_tensor("lin_b", [MEL, 1], F32, kind="ExternalInput").ap()
    out_d = nc.dram_tensor("out_mel", [MEL, 2, CHUNK, 2, B], F32,
                           kind="ExternalOutput").ap()

    with tile.TileContext(nc) as tc, ExitStack() as ctx:
        wpool = ctx.enter_context(tc.tile_pool(name="weights", bufs=1))
        xpool = ctx.enter_context(tc.tile_pool(name="xstream", bufs=4))
        state = ctx.enter_context(tc.tile_pool(name="state", bufs=3))
        actp = ctx.enter_context(tc.tile_pool(name="acts", bufs=3))
        xbig = ctx.enter_context(tc.tile_pool(name="xbig", bufs=1))
        scr = ctx.enter_context(tc.tile_pool(name="scratch", bufs=3))
        psA = [ctx.enter_context(tc.tile_pool(name=f"gA{i}", bufs=1,
                                              space="PSUM")) for i in range(4)]
        psB = [ctx.enter_context(tc.tile_pool(name=f"gB{i}", bufs=1,
                                              space="PSUM")) for i in range(4)]
        ostage = ctx.enter_context(tc.tile_pool(name="ostage", bufs=2))

        # ---- memsets first (Pool queue) so the PE pre-warm starts at t~0
        hinit = wpool.tile([128, 256], BF16, tag="hinit")
        nc.gpsimd.memset(hinit[:], 0.0)
        zstat_bf = wpool.tile([128, 64], BF16, tag="zstatbf")
        nc.gpsimd.memset(zstat_bf[:], 0.0)

        # PE p-state pre-warm: burn the ramp on dummy matmuls while the
        # weight DMAs are in flight, so step 0 runs at full clock.
        warm = psB[3].tile([128, 512], F32, tag="g", name="pewarm")
        NWARM = 10
        for i in range(NWARM):
            nc.tensor.matmul(warm[0:64, 0:256], zstat_bf[:], hinit[:],
                             start=(i == 0), stop=(i == NWARM - 1))

        # ---- xe stream DMAs ----
        xe_tiles = {}

        def emit_dma(s):
            if s not in xe_tiles and s < K_STEPS:
                xe = xpool.tile([EMB, 512], BF16, tag="xe", name=f"xe{s}")
                nc.sync.dma_start(xe[:], xein[s])
                xe_tiles[s] = xe

        emit_dma(0)
        emit_dma(1)

        # ---- weights -> SBUF
        wih_f = wpool.tile([128, 8 * 128], BF16, tag="wihf")
        nc.scalar.dma_start(wih_f[:], wih_f_d[:])
        wih_b = wpool.tile([128, 8 * 128], BF16, tag="wihb")
        nc.gpsimd.dma_start(wih_b[:], wih_b_d[:])
        whh_f = wpool.tile([128, 16 * 128], BF16, tag="whhf")
        nc.sync.dma_start(whh_f[:], whh_f_d[:])
        whh_b = wpool.tile([128, 16 * 128], BF16, tag="whhb")
        nc.scalar.dma_start(whh_b[:], whh_b_d[:])
        lin_w = wpool.tile([128, 4 * MEL], BF16, tag="linw")
        nc.scalar.dma_start(lin_w[:], lin_w_d[:])
        lin_b = wpool.tile([MEL, 1], F32, tag="linb")
        nc.gpsimd.dma_start(lin_b[:], lin_b_d[:])

        XW = (CHUNK + 2) * 128
        Xs = [xbig.tile([128, 2 * XW], BF16, tag=f"X{i}", name=f"X{i}")
              for i in range(4)]

        # chains: FA, BA, FB, BB (emit order); xe col blocks FA 0:128,
        # FB 128:256, BA 256:384, BB 384:512
        chains = [
            _Chain("FA", whh_f, wih_f, slice(0, 128), Xs[0], psA[0], psB[0]),
            _Chain("BA", whh_b, wih_b, slice(256, 384), Xs[1], psA[1], psB[1]),
            _Chain("FB", whh_f, wih_f, slice(128, 256), Xs[2], psA[2], psB[2]),
            _Chain("BB", whh_b, wih_b, slice(384, 512), Xs[3], psA[3], psB[3]),
        ]
        for ch in chains:
            ch.src = (hinit[:, 0:128], hinit[:, 128:256])
            c0 = state.tile([128, 256], BF16, tag="c" + ch.name,
                            name=f"c0{ch.name}")
            nc.gpsimd.memset(c0[:], 0.0)
            ch.c_prev = c0

        def emit_mms(ch, s):
            # all of bank A (xe + rec, k-major), then bank B. bufs=1: the
            # tile from step s-1 is recycled; Tile waits on its readers.
            xe = xe_tiles[s]
            gA = ch.poolA.tile([128, 512], F32, tag="g", name=f"gA{ch.name}{s}")
            gB = ch.poolB.tile([128, 512], F32, tag="g", name=f"gB{ch.name}{s}")
            for g, mbase in ((gA, 0), (gB, 4)):
                for m in range(mbase, mbase + 4):
                    col = (m % 4) * 128
                    nc.tensor.matmul(g[:, col:col + 128],
                                     ch.wih[:, m * 128:(m + 1) * 128],
                                     xe[:, ch.xe_cols],
                                     start=(m == mbase),
                                     stop=(s == 0 and m == mbase + 3))
                if s > 0:
                    for k in (0, 1):
                        for m in range(mbase, mbase + 4):
                            last = (m == mbase + 3) and (k == 1)
                            col = (m % 4) * 128
                            nc.tensor.matmul(
                                g[:, col:col + 128],
                                ch.whh[:, (2 * m + k) * 128:(2 * m + k + 1) * 128],
                                ch.src[k],
                                start=False, stop=last)
            ch.gA, ch.gB = gA, gB

        # ---- phase 2 (final linear) groups per chain pair ----
        movs = {}
        for p, (Xf, Xb) in enumerate(((Xs[0], Xs[1]), (Xs[2], Xs[3]))):
            movs[p] = [X[:, k * XW:k * XW + CHUNK * 128].rearrange(
                           "p (t l) -> p t l", l=128)
                       for X in (Xf, Xb) for k in (0, 1)]
        gstate = {"gi": 0}

        def emit_group(p, p0, glen):
            n = glen * 128
            ps = psA[gstate["gi"] % 2].tile([MEL, 512], F32, tag="g",
                                            name=f"op{p}_{p0}")
            for k in range(4):
                nc.tensor.matmul(ps[:, 0:n], lin_w[:, k * MEL:(k + 1) * MEL],
                                 movs[p][k][:, p0:p0 + glen],
                                 start=(k == 0), stop=(k == 3))
            o_sb = ostage.tile([MEL, 512], F32, tag="os", name=f"os{p}_{p0}")
            nc.vector.tensor_scalar(o_sb[:, 0:n], ps[:, 0:n], lin_b[:], None,
                                    ADD)
            q = (nc.sync, nc.gpsimd)[gstate["gi"] % 2]
            q.dma_start(out_d[:, p, p0:p0 + glen], o_sb[:, 0:n])
            gstate["gi"] += 1

        groups_at = {}
        for p in (0, 1):
            p0 = 0
            while p0 < CHUNK:
                glen = min(4, CHUNK - p0)
                ready = W + max(p0 + glen - 1, CHUNK - 1 - p0)
                groups_at.setdefault(min(ready + (p % 2), K_STEPS - 1),
                                     []).append((p, p0, glen))
                p0 += glen

        def emit_sf(ch, s):
            sf = actp.tile([128, 512], BF16, tag="sf" + ch.name,
                           name=f"sf{ch.name}{s}")
            nc.scalar.activation(sf[:], ch.gA[:], SIG)
            ch.sf = sf

        def emit_tB(ch, s):
            t = actp.tile([128, 512], BF16, tag="tB" + ch.name,
                          name=f"tB{ch.name}{s}")
            nc.scalar.activation(t[:], ch.gB[:], TANH, scale=0.5)
            ch.tB = t

        def emit_fc(ch, s):
            fc = scr.tile([128, 256], BF16, tag="fc" + ch.name,
                          name=f"fc{ch.name}{s}")
            nc.vector.tensor_mul(fc[:], ch.sf[:, 256:512], ch.c_prev[:])
            ch.fc = fc

        def emit_ig(ch, s):
            ig = scr.tile([128, 256], BF16, tag="ig" + ch.name,
                          name=f"ig{ch.name}{s}")
            nc.vector.tensor_mul(ig[:], ch.sf[:, 0:256], ch.tB[:, 0:256])
            ch.ig = ig

        cpair = {}

        def emit_cnew(ch, s, pair, side):
            # both chains of a pair write one [128,512] c tile so tanh(c)
            # runs as ONE merged act per pair.
            if side == 0:
                cpair[pair] = state.tile([128, 512], BF16, tag=f"cP{pair}",
                                         name=f"cP{pair}_{s}")
            c_new = cpair[pair][:, side * 256:(side + 1) * 256]
            nc.vector.tensor_add(c_new, ch.fc[:], ch.ig[:])
            ch.c_new = c_new

        def emit_tc_pair(pair, s, ch_a, ch_b):
            tc_ = actp.tile([128, 512], BF16, tag=f"tcP{pair}",
                            name=f"tcP{pair}_{s}")
            nc.scalar.activation(tc_[:], cpair[pair][:], TANH)
            ch_a.tc = tc_[:, 0:256]
            ch_b.tc = tc_[:, 256:512]

        def emit_h(ch, s):
            real = s >= W
            t_rel = s - W
            if real:
                lp = t_rel if ch.name[0] == "F" else CHUNK - 1 - t_rel
            else:
                lp = CHUNK + (s & 1)
            dst = tuple(ch.X[:, k * XW + lp * 128:k * XW + (lp + 1) * 128]
                        for k in (0, 1))
            # h2 = (tanh(o/2) + 1) * tanh(c) = 2*sigma(o)*tanh(c); k-half
            # writes so next step's k0 recurrent mms start after half lands.
            for k in (0, 1):
                nc.vector.scalar_tensor_tensor(
                    dst[k], ch.tB[:, 256 + k * 128:256 + (k + 1) * 128], 1.0,
                    ch.tc[:, k * 128:(k + 1) * 128], ADD, MULT)
            ch.src = dst
            ch.c_prev = ch.c_new

        for s in range(K_STEPS):
            emit_dma(s + 1)
            for ch in chains:
                emit_mms(ch, s)
            emit_dma(s + 2)

            # Act queue: sfFA tBFA sfBA tBBA tcP0 sfFB tBFB sfBB tBBB tcP1;
            # DVE trails each chain's acts; h writes follow the pair's tc.
            c0, c1, c2, c3 = chains
            emit_sf(c0, s)
            emit_tB(c0, s)
            emit_sf(c1, s)
            emit_fc(c0, s)
            emit_ig(c0, s)
            emit_cnew(c0, s, 0, 0)
            emit_tB(c1, s)
            emit_fc(c1, s)
            emit_ig(c1, s)
            emit_cnew(c1, s, 0, 1)
            emit_tc_pair(0, s, c0, c1)
            emit_sf(c2, s)
            emit_tB(c2, s)
            emit_fc(c2, s)
            emit_ig(c2, s)
            emit_cnew(c2, s, 1, 0)
            emit_h(c0, s)
            emit_h(c1, s)
            emit_sf(c3, s)
            emit_tB(c3, s)
            emit_fc(c3, s)
            emit_ig(c3, s)
            emit_cnew(c3, s, 1, 1)
            emit_tc_pair(1, s, c2, c3)
            emit_h(c2, s)
            emit_h(c3, s)

            for (p, p0g, gl) in groups_at.get(s, []):
                emit_group(p, p0g, gl)

    nc.compile()
    return nc


def _np_lstm_fallback(exp, inputs):
    def sigmoid(z):
        return 1.0 / (1.0 + np.exp(-z))

    def lstm(xs, wih, whh, bih, bhh):
        Bb, L, E = xs.shape
        pre = np.einsum("ble,ge->blg", xs, wih) + bih + bhh
        h = np.zeros((Bb, HID), np.float32)
        c = np.zeros((Bb, HID), np.float32)
        hs = np.zeros((Bb, L, HID), np.float32)
        for t in range(L):
            gg = pre[:, t] + h @ whh.T
            i, f, g_, o = np.split(gg, 4, axis=-1)
            c = sigmoid(f) * c + sigmoid(i) * np.tanh(g_)
            h = sigmoid(o) * np.tanh(c)
            hs[:, t] = h
        return hs

    out_f = lstm(exp, inputs["wih_f"], inputs["whh_f"], inputs["bih_f"],
                 inputs["bhh_f"])
    out_b = lstm(exp[:, ::-1], inputs["wih_b"], inputs["whh_b"],
                 inputs["bih_b"], inputs["bhh_b"])[:, ::-1]
    out = np.concatenate([out_f, out_b], axis=-1)
    return out @ inputs["lin_w"].T + inputs["lin_b"]


def make_in_maps(expP, expR, inputs):
    import ml_dtypes
    bf16 = ml_dtypes.bfloat16
    rows, sc_ih, sc_hh = _mchunk_rows()

    def stat_tiles(w, scale):
        wp = (w.astype(np.float32)[rows] * scale[:, None])
        nk = w.shape[1] // 128
        out = np.zeros((128, 8 * nk * 128), np.float32)
        for m in range(8):
            for k in range(nk):
                out[:, (m * nk + k) * 128:(m * nk + k + 1) * 128] = \
                    wp[m * 128:(m + 1) * 128, k * 128:(k + 1) * 128].T
        return np.ascontiguousarray(out).astype(bf16)

    whhT_f = stat_tiles(inputs["whh_f"], sc_hh)
    whhT_b = stat_tiles(inputs["whh_b"], sc_hh)
    wihT_f = stat_tiles(inputs["wih_f"], sc_ih)
    wihT_b = stat_tiles(inputs["wih_b"], sc_ih)
    lw = inputs["lin_w"].astype(np.float32) * 0.5
    linT = np.concatenate([np.ascontiguousarray(lw[:, k * 128:(k + 1) * 128].T)
                           for k in range(4)], axis=1).astype(bf16)
    lin_b2 = np.ascontiguousarray(inputs["lin_b"].astype(np.float32)[:, None])

    in_maps = []
    for j in range(N_CORES):
        xein = np.zeros((K_STEPS, EMB, 512), np.float32)
        # lane blocks of 64: FA=(4j,4j+1), FB=(4j+2,4j+3),
        # BA=(31-4j,30-4j), BB=(29-4j,28-4j); xe cols FA 0:128, FB 128:256,
        # BA 256:384, BB 384:512
        cks = [4 * j, 4 * j + 1, 4 * j + 2, 4 * j + 3,
               31 - 4 * j, 30 - 4 * j, 29 - 4 * j, 28 - 4 * j]
        srcs = [expP] * 4 + [expR] * 4
        for s in range(K_STEPS):
            for ci, (ck, src) in enumerate(zip(cks, srcs)):
                p = ck * CHUNK - W + s
                if 0 <= p < L_PAD:
                    xein[s, :, ci * 64:(ci + 1) * 64] = src[:, p].T
        in_maps.append({
            "xein": xein.astype(bf16),
            "whhT_f": whhT_f, "whhT_b": whhT_b,
            "wihT_f": wihT_f, "wihT_b": wihT_b,
            "linT": linT, "lin_b": lin_b2,
        })
    return in_maps


def kernel(**inputs):
    global _COMPILED
    inputs = {k: np.asarray(v) for k, v in inputs.items()}
    x = inputs["x"].astype(np.int64)
    exp, L = _host_expand(x, inputs["embed"].astype(np.float32),
                          inputs["dp_w"].astype(np.float32),
                          inputs["dp_b"].astype(np.float32))

    bias_mag = max(float(np.abs(inputs[k]).max())
                   for k in ("bih_f", "bhh_f", "bih_b", "bhh_b"))
    if L > L_PAD or bias_mag != 0.0:
        f32in = {k: (v.astype(np.float32) if v.dtype.kind == "f" else v)
                 for k, v in inputs.items()}
        return _np_lstm_fallback(exp, f32in).astype(np.float32)

    expP = np.zeros((B, L_PAD, EMB), np.float32)
    expP[:, :L] = exp
    expR = expP[:, ::-1]

    in_maps = make_in_maps(expP, expR, inputs)

    if _COMPILED is None:
        _COMPILED = _build_kernel()
    nc = _COMPILED

    res = run_bass_kernel_spmd(nc, in_maps, core_ids=list(range(N_CORES)))

    out = np.empty((B, L_PAD, MEL), np.float32)
    for j in range(N_CORES):
        om = res.results[j]["out_mel"]          # [MEL, 2, CHUNK, 2, B]
        for p in (0, 1):
            for half in (0, 1):
                seg = om[:, p, :, half, :]      # [MEL, CHUNK, B]
                c = 4 * j + 2 * p + half
                out[:, c * CHUNK:(c + 1) * CHUNK] = seg.transpose(2, 1, 0)
    return np.ascontiguousarray(out[:, :L])


if __name__ == "__main__":
    inputs = dict(np.load("/root/problem/inputs.npz"))
    out = kernel(**inputs)
    ref = np.load("/root/problem/expected.npy")
    diff = np.abs(out - ref)
    print("out", out.shape, "absmax diff", diff.max(),
          "rel", diff.max() / np.abs(ref).max())
